# revision 1
# baseline (speedup 1.0000x reference)
"""Trainium2 Bass kernel: single-head causal attention.

Problem: x[4,4096,128]; Q/K/V linear projections (W [in,out] layout, +bias);
scores = QK^T/sqrt(128) with causal mask; softmax; out = P @ V.

Sharding (8 cores = 4 batches x 2): every core runs the SAME program
(SPMD requirement) on different data:
  core (b, h):
    triangle part: queries q in [2048h, 2048h+2048) of batch b attending
        causally to kv rows in the same range (relative causal structure is
        identical for h=0 and h=1).
    rectangle part: queries q in [2048, 4096) of batch b attending to kv rows
        [1024h, 1024h+1024)  (fully valid, no mask, since kv < 2048 <= q).
  Union over both cores of a batch covers the full causal set exactly once.

Softmax is computed WITHOUT max subtraction (scores are ~N(0,1) by
construction: Wq is pre-scaled by 1/sqrt(128) on host, so exp never
overflows), which makes the cross-core merge linear: the host sums
unnormalized outputs o and denominators l, then divides.

Bias handling:
  - bk drops out of softmax entirely (adds a per-query constant to scores).
  - bq is pre-scaled on host and added to Q^T during the PSUM->SBUF copy
    (per-partition scalar add on the vector engine).
  - bv is added on the host after normalization (rows of P sum to 1).

Matmuls run in float32r (TF32-like: fp32 storage, 11-bit mantissa, full PE
rate at moving free dim >= 256). The BIR verifier requires every producer of
an f32r matmul operand to emit f32r (hardware rounds on write); host-side
inputs are pre-rounded with the exact RNE-to-11-bits rule.

Device layouts (per core):
  xTq [128,4096]  x^T columns for this core's 4096 query slots (tri|rect)
  xTk [128,3072]  x^T columns for kv rows (tri 2048 | rect 1024)
  QT = (x@Wq')^T + bq'  [128(e), 4096(q)]   (e on partitions)
  KT = (x@Wk)^T         [128(e), 3072(k)]
  V  = x@Wv    as 24 tiles [128(kv row), 128(e)] packed in [128, 3072]
  Scores are computed TRANSPOSED: ST[k, q] = K Q^T (PSUM), masked on
  diagonal tiles, exp'd on the scalar engine into P~T [k, q] (SBUF).
  AV:  oT[e, q] += V_t^T-matmul-P~T   (accumulated in PSUM over kv tiles)
  l:   l[q]    += ones-matmul-P~T     (PE is the only partition reducer)
Outputs: oT [128, 4096] (transposed, unnormalized), lv [8,512] (denominators
per 512-query chunk). Host transposes, merges, normalizes, adds bv.
"""

import math
import sys

import numpy as np

sys.path.insert(0, "/opt/trn_rl_repo")

import concourse.bass as bass  # noqa: E402
import concourse.mybir as mybir  # noqa: E402
from concourse.tile import TileContext  # noqa: E402

B, T, D = 4, 4096, 128
HALF = T // 2          # 2048 queries per triangle
NCHUNK = 8             # 8 chunks of 512 query slots per core (4 tri + 4 rect)
CHUNK = 512
KV_TRI_TILES = 16      # triangle kv tiles (2048 rows)
KV_RECT_TILES = 8      # rectangle kv tiles (1024 rows)
KV_TILES = KV_TRI_TILES + KV_RECT_TILES          # 24 tiles = 3072 kv rows
NEG = -1.0e5           # additive mask value; exp(NEG) == 0.0 in fp32

F32 = mybir.dt.float32
F32R = mybir.dt.float32r


def round_f32r(a):
    """Exact fp32 -> fp32r rounding (RNE to 11 mantissa bits), matching
    walrus fp32_to_fp32r."""
    u = np.ascontiguousarray(a, np.float32).view(np.uint32)
    add = np.uint32(0x7FF) + ((u >> np.uint32(12)) & np.uint32(1))
    return ((u + add) & np.uint32(0xFFFFF000)).view(np.float32)


def build_nc(legalize=True):
    nc = bass.Bass()

    xtq_d = nc.declare_dram_parameter("xTq", [D, T], F32R, isOutput=False)
    xtk_d = nc.declare_dram_parameter("xTk", [D, KV_TILES * 128], F32R, isOutput=False)
    wq_d = nc.declare_dram_parameter("Wqs", [D, D], F32R, isOutput=False)
    wk_d = nc.declare_dram_parameter("Wk", [D, D], F32R, isOutput=False)
    wv_d = nc.declare_dram_parameter("Wv", [D, D], F32R, isOutput=False)
    bq_d = nc.declare_dram_parameter("bqs", [D], F32, isOutput=False)
    msk_d = nc.declare_dram_parameter("msk", [4, D, CHUNK], F32R, isOutput=False)
    ident_d = nc.declare_dram_parameter("ident", [D, D], F32R, isOutput=False)
    ones_d = nc.declare_dram_parameter("ones", [D, 1], F32R, isOutput=False)

    ot_d = nc.declare_dram_parameter("oT", [D, T], F32, isOutput=True)
    lv_d = nc.declare_dram_parameter("lv", [NCHUNK, CHUNK], F32, isOutput=True)

    with TileContext(nc) as tc:
        with (
            tc.tile_pool(name="big", bufs=1) as big,
            tc.tile_pool(name="small", bufs=1) as small,
        ):
            # ---- resident SBUF tensors: first-consumed DMAs first (the
            # V projection needs wv + xtk chunk 0 before anything else) ----
            wv = small.tile([D, D], F32R)
            nc.sync.dma_start(out=wv, in_=wv_d[:, :])
            xtk = big.tile([D, KV_TILES * 128], F32R)
            nc.sync.dma_start(out=xtk[:, 0:CHUNK], in_=xtk_d[:, 0:CHUNK])
            wk = small.tile([D, D], F32R)
            nc.sync.dma_start(out=wk, in_=wk_d[:, :])
            wq = small.tile([D, D], F32R)
            nc.sync.dma_start(out=wq, in_=wq_d[:, :])
            bq = small.tile([D, 1], F32)
            nc.sync.dma_start(out=bq, in_=bq_d[:].unsqueeze(1))
            ones = small.tile([D, 1], F32R)
            nc.sync.dma_start(out=ones, in_=ones_d[:, :])
            for j in range(1, KV_TILES * 128 // CHUNK):
                sl = slice(j * CHUNK, (j + 1) * CHUNK)
                nc.sync.dma_start(out=xtk[:, sl], in_=xtk_d[:, sl])
            xtq = big.tile([D, T], F32R)
            for j in range(T // 1024):
                sl = slice(j * 1024, (j + 1) * 1024)
                nc.sync.dma_start(out=xtq[:, sl], in_=xtq_d[:, sl])
            ident = small.tile([D, D], F32R)
            nc.sync.dma_start(out=ident, in_=ident_d[:, :])
            msk = big.tile([D, 4 * CHUNK], F32R)
            nc.sync.dma_start(
                out=msk.rearrange("p (m q) -> p m q", m=4),
                in_=msk_d[:, :, :].transpose([1, 0, 2]),
            )

            qt = big.tile([D, T], F32R)               # Q^T (scaled, biased)
            kt = big.tile([D, KV_TILES * 128], F32R)  # K^T
            vsb = big.tile([D, KV_TILES * 128], F32R)  # V tiles [kvrow, e]

            # The ST pool is opened FIRST so the stack allocator gives it
            # PSUM banks the projection phase never touches: the first
            # attention score matmuls then carry no release deps from the
            # projection pools and overlap the projection tail on the PE.
            stp_cm = tc.tile_pool(name="stp", bufs=2, space="PSUM")
            stp = stp_cm.__enter__()
            # ---- projections (order: V, K, Q so the DVE tick PE waits on
            # for qt also covers vsb/kt; "touch" matmuls absorb each DMA
            # semaphore into PE's clock first, because the fused-weight-load
            # fp32r matmul instruction supports only ONE sync wait) ----
            with (
                tc.tile_pool(name="ppsum", bufs=1, space="PSUM")) as ppsum:
                # (the former "touch" matmuls that absorbed DMA semaphores
                # into PE's clock are gone: the post-Tile wait legalizer
                # handles multi-wait instructions directly, and dropping
                # them frees their PSUM bank for a 4-deep projection
                # rotation plus ~2us of PE dispatch)

                # Pool-recycled PSUM tiles hand every accessor of the new
                # tile the old tile's full release deps (PE write + DVE read)
                # - 2 sync waits, over the fused-weight-load fp32r matmul
                # limit of 1. A single persistent 3-bank tile with manual
                # region rotation keeps deps intra-tile: same-engine WAW is
                # program-order (no sem), so each matmul carries only the
                # DVE WAR wait.
                pps = [ppsum.tile([D, CHUNK], F32, name=f"pps{s}")
                       for s in range(4)]
                nps = [0]

                def proj_ps():
                    s = nps[0] % 4
                    nps[0] += 1
                    return pps[s], s

                for g in range(KV_TILES // 4):     # V: 24 tiles, batched 4/bank
                    ps, s = proj_ps()
                    for jj in range(4):
                        t = 4 * g + jj
                        nc.tensor.matmul(
                            ps[:, jj * 128:(jj + 1) * 128],
                            xtk[:, t * 128:(t + 1) * 128], wv,
                            start=True, stop=True, skip_group_check=True,
                        )
                    if g % 2 == 0:
                        nc.vector.tensor_copy(
                            vsb[:, g * CHUNK:(g + 1) * CHUNK], ps)
                    else:
                        nc.scalar.copy(vsb[:, g * CHUNK:(g + 1) * CHUNK], ps)
                for j in range(KV_TILES * 128 // CHUNK):   # K^T: 6 chunks
                    ps, s = proj_ps()
                    nc.tensor.matmul(
                        ps, wk, xtk[:, j * CHUNK:(j + 1) * CHUNK],
                        start=True, stop=True, skip_group_check=True,
                    )
                    if j % 2 == 0:
                        nc.vector.tensor_copy(
                            kt[:, j * CHUNK:(j + 1) * CHUNK], ps)
                    else:
                        nc.scalar.copy(kt[:, j * CHUNK:(j + 1) * CHUNK], ps)
                for j in range(T // CHUNK):        # Q^T: 8 chunks
                    ps, s = proj_ps()
                    nc.tensor.matmul(
                        ps, wq, xtq[:, j * CHUNK:(j + 1) * CHUNK],
                        start=True, stop=True, skip_group_check=True,
                    )
                    if j % 2 == 0:
                        nc.vector.tensor_scalar_add(
                            qt[:, j * CHUNK:(j + 1) * CHUNK], ps, bq)
                    else:
                        nc.scalar.activation(
                            qt[:, j * CHUNK:(j + 1) * CHUNK], ps,
                            mybir.ActivationFunctionType.Identity, bias=bq)
                # final pump: absorb the last DVE copies before attention

            # ---- attention: 8 chunks, kv-tile pairs, software-pipelined ----
            # chunk c covers query slots [512c, 512c+512).
            # tri chunks (0-3): kv tiles 0..4c+3; rect chunks (4-7): 16..23.
            # Pairs are processed in REVERSE kv order so the diagonal
            # (masked) pairs land at chunk starts, where the previous
            # chunk's AV/l matmuls hide the mask-add + exp latency.
            # The AV+l matmuls of unit u are emitted after ST/exp of unit
            # u+1 (skew-1 software pipeline) so PE never waits on ACT.
            # Tri chunks: the 4 diagonal tiles first in ASCENDING m order
            # (so the first AV/l matmul of the chunk covers the full column
            # range with start=True and later sliced matmuls only ever
            # accumulate onto initialized columns), then the full tiles.
            chunk_ts = [list(range(4 * c, 4 * c + 4)) +
                        list(range(0, 4 * c))[::-1] for c in range(4)] + \
                       [list(range(16, 24))[::-1] for _ in range(4)]
            units = []
            for c, ts in enumerate(chunk_ts):
                pairs = [ts[i:i + 2] for i in range(0, len(ts), 2)]
                for pi, pair in enumerate(pairs):
                    units.append((c, ts, pair, pi == len(pairs) - 1))
            with (
                tc.tile_pool(name="op", bufs=2, space="PSUM") as op,
                tc.tile_pool(name="lp", bufs=2, space="PSUM") as lp,
                tc.tile_pool(name="ptp", bufs=1) as ptp,
                tc.tile_pool(name="osb", bufs=8) as osb,
                tc.tile_pool(name="lsb", bufs=8) as lsb,
            ):
                pts = [ptp.tile([D, 2 * CHUNK], F32R, name=f"pt{i}")
                       for i in range(3)]
                npt = [0]
                acc = {}                # chunk -> (po, pl)
                pending = None          # (c, ts, pair, is_last, pt)
                epiq = []               # delayed epilogues [(c, po, pl)]

                def emit_epilogue():
                    c, po, pl = epiq.pop(0)
                    # epilogue copies on ACT (scalar): the PSUM-slot WAR
                    # dependency of a later chunk's first AV matmul then
                    # consolidates onto the ACT semaphore (1-wait limit).
                    # Delayed one pipeline unit so these ACT ops never sit
                    # between an ST matmul and the exp PE is waiting for.
                    qsl = slice(c * CHUNK, (c + 1) * CHUNK)
                    ob = osb.tile([D, CHUNK], F32, tag="ob", name="ob")
                    nc.vector.tensor_copy(ob, po)
                    nc.sync.dma_start(out=ot_d[:, qsl], in_=ob)
                    lb = lsb.tile([1, CHUNK], F32, tag="lb", name="lb")
                    nc.vector.tensor_copy(lb, pl)
                    nc.sync.dma_start(out=lv_d[c:c + 1, :], in_=lb)

                def emit_av(pend):
                    c, ts, pair, is_last, pt, los = pend
                    if c not in acc:
                        acc[c] = (
                            op.tile([D, CHUNK], F32, tag="po", name="po"),
                            lp.tile([1, CHUNK], F32, tag="pl", name="pl"),
                        )
                    po, pl = acc[c]
                    qsl = slice(c * CHUNK, (c + 1) * CHUNK)
                    for i, t in enumerate(pair):
                        lo = los[i]
                        ptc = pt[:, i * CHUNK + lo:(i + 1) * CHUNK]
                        nc.tensor.matmul(
                            po[:, lo:], vsb[:, t * 128:(t + 1) * 128], ptc,
                            start=(t == ts[0]), stop=(t == ts[-1]),
                            skip_group_check=True,
                        )
                        nc.tensor.matmul(
                            pl[0:1, lo:], ones, ptc,
                            start=(t == ts[0]), stop=(t == ts[-1]),
                            skip_group_check=True,
                        )
                    if is_last:
                        epiq.append((c, po, pl))
                        del acc[c]

                for c, ts, pair, is_last in units:
                    if epiq:
                        emit_epilogue()
                    # Diagonal sub-tile m: every score column q' < 128m is
                    # fully masked (q' < 128m <= 128m + k for all k), so the
                    # ST / mask / exp / AV / l work all skip that prefix.
                    # Within the remaining window only the 128-column band
                    # [128m, 128(m+1)) needs the staircase mask.
                    los = [128 * (t - 4 * c) if c < 4 and t >= 4 * c else 0
                           for t in pair]
                    st = stp.tile([D, 2 * CHUNK], F32, tag="st", name="st")
                    for i, t in enumerate(pair):
                        lo = los[i]
                        nc.tensor.matmul(
                            st[:, i * CHUNK + lo:(i + 1) * CHUNK],
                            kt[:, t * 128:(t + 1) * 128],
                            qt[:, c * CHUNK + lo:(c + 1) * CHUNK],
                            start=True, stop=True, skip_group_check=True,
                        )
                        if c < 4 and t >= 4 * c:
                            m = t - 4 * c
                            nc.tensor.matmul(
                                st[:, i * CHUNK + lo:i * CHUNK + lo + 128],
                                ident,
                                msk[:, m * CHUNK + lo:m * CHUNK + lo + 128],
                                start=False, stop=True, skip_group_check=True,
                            )
                    pt = pts[npt[0] % 3]
                    npt[0] += 1
                    if len(pair) == 2 and los[1] > 0:
                        # sliced halves with an uninitialized gap: exp each
                        # half's valid window separately
                        nc.scalar.activation(
                            pt[:, los[0]:CHUNK], st[:, los[0]:CHUNK],
                            mybir.ActivationFunctionType.Exp,
                        )
                        nc.scalar.activation(
                            pt[:, CHUNK + los[1]:], st[:, CHUNK + los[1]:],
                            mybir.ActivationFunctionType.Exp,
                        )
                    else:
                        nc.scalar.activation(
                            pt[:, los[0]:], st[:, los[0]:],
                            mybir.ActivationFunctionType.Exp,
                        )
                    prev, pending = pending, (c, ts, pair, is_last, pt, los)
                    if prev is not None:
                        emit_av(prev)
                emit_av(pending)
                while epiq:
                    emit_epilogue()
            stp_cm.__exit__(None, None, None)

    if legalize:
        _legalize_multiwaits(nc)
    nc.finalize()
    return nc


def _legalize_multiwaits(nc):
    """Hardware instruction structs in this walrus build accept at most ONE
    sync wait. For any instruction left with >= 2 waits after Tile's sem
    assignment, move all but the last wait onto single-wait same-engine
    NoOps inserted right before it. Engines execute in order, so waiting
    earlier on the same engine preserves semantics exactly.
    """
    for fn in nc.m.functions:
        for blk in fn.blocks:
            insts = blk.instructions
            out = []
            for inst in insts:
                si = inst.sync_info
                if si is not None and si.on_wait and len(si.on_wait) >= 2:
                    waits = list(si.on_wait)
                    for w in waits[:-1]:
                        out.append(mybir.InstNoOp(
                            name=nc.get_next_instruction_name(),
                            engine=inst.engine,
                            bass_nofuse=True,
                            sync_info=mybir.SyncInfo(
                                on_wait=[w], on_update=[]),
                        ))
                    inst.sync_info = mybir.SyncInfo(
                        on_wait=[waits[-1]],
                        on_update=list(si.on_update or []))
                out.append(inst)
            insts[:] = out


_NC_CACHE = {}


def get_nc(legalize=True):
    key = ("nc", legalize)
    if key not in _NC_CACHE:
        _NC_CACHE[key] = build_nc(legalize)
    return _NC_CACHE[key]


def make_core_inputs(x, Wq, bq, Wk, bk, Wv, bv):
    """Per-core input maps (host-side sharding). bk is dropped (softmax
    invariance); bv is applied on the host. f32r-consumed inputs are
    pre-rounded to match the hardware's assumed rounding."""
    s = 1.0 / math.sqrt(D)
    wq_s = round_f32r(np.asarray(Wq, np.float32) * s)
    bq_s = (np.asarray(bq, np.float32) * s).astype(np.float32)
    wk = round_f32r(np.asarray(Wk, np.float32))
    wv = round_f32r(np.asarray(Wv, np.float32))

    # diagonal masks: msk[m][k, q'] = 0 if q' >= 128*m + k else NEG
    qp = np.arange(CHUNK)[None, :]
    kk = np.arange(128)[:, None]
    msk = round_f32r(np.stack(
        [np.where(qp >= 128 * m + kk, 0.0, NEG) for m in range(4)]
    ).astype(np.float32)).reshape(4, D, CHUNK)
    ident = np.eye(D, dtype=np.float32)

    ones = np.ones((D, 1), np.float32)

    x = np.asarray(x, dtype=np.float32)
    in_maps = []
    for core in range(8):
        b, h = core // 2, core % 2
        xb = x[b]                                   # [4096, 128]
        tri = xb[h * HALF:(h + 1) * HALF]           # [2048, 128]
        rect_q = xb[HALF:]                          # [2048, 128]
        rect_kv = xb[h * 1024:(h + 1) * 1024]       # [1024, 128]
        xtq = round_f32r(np.ascontiguousarray(
            np.concatenate([tri, rect_q], axis=0).T))     # [128, 4096]
        xtk = round_f32r(np.ascontiguousarray(
            np.concatenate([tri, rect_kv], axis=0).T))    # [128, 3072]
        in_maps.append({
            "xTq": xtq, "xTk": xtk, "Wqs": wq_s, "Wk": wk, "Wv": wv,
            "bqs": bq_s, "msk": msk, "ones": ones, "ident": ident,
        })
    return in_maps


def merge_outputs(results, bv):
    """Gather per-core (oT, lv) into the full [B, T, D] output."""
    bv = np.asarray(bv, dtype=np.float32)
    out = np.empty((B, T, D), np.float32)
    for b in range(B):
        lo, hi = results[2 * b], results[2 * b + 1]
        O = np.zeros((T, D), np.float64)
        L = np.zeros(T, np.float64)
        O[:HALF] += lo["oT"][:, :HALF].T
        L[:HALF] += lo["lv"][0:4].ravel()
        O[HALF:] += hi["oT"][:, :HALF].T
        L[HALF:] += hi["lv"][0:4].ravel()
        O[HALF:] += lo["oT"][:, HALF:].T
        L[HALF:] += lo["lv"][4:8].ravel()
        O[HALF:] += hi["oT"][:, HALF:].T
        L[HALF:] += hi["lv"][4:8].ravel()
        out[b] = (O / L[:, None]).astype(np.float32) + bv
    return out


def run_per_core(nc, in_maps, threads=True):
    """Run the same single-core program on each NeuronCore with its own
    inputs. The multi-core shard_map path in run_bass_via_pjrt stalls under
    this container's axon tunnel; independent single-device dispatches work
    (the cores share no collectives, so per-core dispatch is equivalent)."""
    import jax
    from concourse import bass2jax

    devices = jax.devices()[:len(in_maps)]

    def one(i):
        with jax.default_device(devices[i]):
            return bass2jax.run_bass_via_pjrt(nc, [in_maps[i]], n_cores=1)[0]

    if threads:
        from concurrent.futures import ThreadPoolExecutor
        # warm the compile cache once to avoid 8 racing neuronxcc compiles
        first = one(0)
        with ThreadPoolExecutor(max_workers=7) as ex:
            rest = list(ex.map(one, range(1, len(in_maps))))
        return [first] + rest
    return [one(i) for i in range(len(in_maps))]


def kernel(x, Wq, bq, Wk, bk, Wv, bv, _trace=False):
    from concourse.bass_utils import axon_active, run_bass_kernel_spmd

    nc = get_nc()
    in_maps = make_core_inputs(x, Wq, bq, Wk, bk, Wv, bv)
    if axon_active():
        # This container tunnels devices through axon; the 8-device
        # shard_map dispatch stalls there, so dispatch per-core.
        results = run_per_core(nc, in_maps)
    else:
        # Native /dev/neuron*: the production NrtSession path.
        res = run_bass_kernel_spmd(nc, in_maps, list(range(8)), trace=_trace)
        kernel.last_result = res
        results = res.results
    out = merge_outputs(results, bv)
    return out



# revision 26
# speedup vs baseline: 1.3719x; 1.3719x over previous
"""Trainium2 Bass kernel: single-head causal attention (v2).

Problem: x[4,4096,128]; Q/K/V linear projections (W [in,out] layout, +bias);
scores = QK^T/sqrt(128) with causal mask; softmax; out = P @ V.

Sharding (8 cores = 4 batches x 2), SPMD (all cores run one program):
  core (b, h):
    triangle: queries q in [2048h, 2048h+2048) of batch b attending causally
        to kv rows in the same range.
    rectangle: queries q in [2048, 4096) attending to kv [1024h, 1024h+1024)
        (fully valid, no mask).
  Union over both cores of a batch covers the full causal set exactly once.

Softmax without max subtraction but with a constant shift: every score gets
-4.0 before exp (softmax shift-invariance keeps O/l exact), which brings
P = exp(s-4) into fp8 e4m3 range (max ~6 << 240). The cross-core merge stays
linear: host sums unnormalized o and denominators l, then divides.

Numerics by stage (validated to 6.8e-3 rel err vs the fp32 reference):
  - x and W DMA'd in bf16; projections are bf16 matmuls (1 cycle/row always,
    no fp32r narrow-output penalty), PSUM fp32.
  - QT/KT stored f32r; score matmuls (ST) run fp32r (full rate >= 256 wide).
  - Full (unmasked) kv-tile pairs: exp writes P~T in fp8 e4m3; AV and the
    denominator matmul run in DoubleRow perf mode (two 128-kv tiles per
    pass, 0.5 cycles/row = 4x the fp32r rate). V kept in fp8 (vsb8).
  - Diagonal (causally masked) pairs stay f32r end to end: short softmax
    rows have no error averaging, fp8 there fails the gate (5.8e-2).
    Tile m=3 is widened to a 256-col window (fully-masked prefix) so every
    diagonal ST/AV/l matmul is >= 256 wide (no fp32r 4x narrow penalty).
  - bk drops out of softmax; bq (pre-scaled) added during the Q PSUM->SBUF
    copy; bv added by the host after normalization.

Schedule: projections are software-pipelined INTO the attention chunk
stream (one or two projection matmuls between attention units) so the
scalar engine starts exp'ing ~3.5us in and never starves; the exp stream
(~36us on ACT) is the critical resource, with PE at ~33us underneath it.
Mask adds are bf16 identity-matmuls on the PE (no ACT/DVE time). PSUM->
SBUF copies and output staging are split between DVE and GPSIMD(Pool),
keeping ACT exp-only. PSUM banks: st 2x2 + proj 2x1 + po 1 + pl 1 = 8.

Device layouts (per core):
  xin  [128,5120] bf16  x^T cols: 0:4096 = (tri|rect_q), 4096:5120 = rect_kv
  cb16 [128,768]  bf16  wq|wk|wv|msk(256)|ident
  cf32 [128,4]    f32r  bq' | -4.0 | 1.0 (ones) | fp8-ones bit pattern
  QT [128,4096] f32r; KT [128,3072] f32r
  vsb32 [128,2048] f32r (V tiles 0-15, diag pairs)
  vsb8  [128,3072] fp8  (V tiles 0-23, full pairs)
  ST computed transposed: ST[k,q] = K Q^T in PSUM; exp(ST-4) -> P~T in
  fp8 (full pairs) or f32r (diag pairs); AV: oT[e,q] += V^T-mm-P~T;
  l[q] += ones-mm-P~T (PE is the only partition reducer).
Outputs: oT [128,4096] f32 (transposed, unnormalized), lv [1,4096]
(denominators, chunk-major). Host transposes, merges, normalizes, adds bv.
"""

import math
import sys

import numpy as np

sys.path.insert(0, "/opt/trn_rl_repo")

import concourse.bass as bass  # noqa: E402
import concourse.mybir as mybir  # noqa: E402
from concourse.tile import TileContext  # noqa: E402

B, T, D = 4, 4096, 128
HALF = T // 2          # 2048 queries per triangle
NCHUNK = 8             # 8 chunks of 512 query slots (4 tri + 4 rect)
CHUNK = 512
KV_TILES = 24          # 16 tri + 8 rect kv tiles of 128 rows
NEG = -99840.0         # additive mask value, exactly representable in bf16
SHIFT = 4.0            # score shift: exp(s - 4) keeps P in fp8 range

F32 = mybir.dt.float32
F32R = mybir.dt.float32r
BF16 = mybir.dt.bfloat16
F8 = mybir.dt.float8e4
DR = mybir.MatmulPerfMode.DoubleRow


def round_f32r(a):
    """Exact fp32 -> fp32r rounding (RNE to 11 mantissa bits)."""
    u = np.ascontiguousarray(a, np.float32).view(np.uint32)
    add = np.uint32(0x7FF) + ((u >> np.uint32(12)) & np.uint32(1))
    return ((u + add) & np.uint32(0xFFFFF000)).view(np.float32)


def build_nc(legalize=True):
    nc = bass.Bass()

    xin_d = nc.declare_dram_parameter("xin", [D, 5120], BF16, isOutput=False)
    cb16_d = nc.declare_dram_parameter("cb16", [D, 768], BF16, isOutput=False)
    cf32_d = nc.declare_dram_parameter("cf32", [D, 132], F32R, isOutput=False)
    ones8_d = nc.declare_dram_parameter("ones8", [D, 2 * D], F8, isOutput=False)

    ot_d = nc.declare_dram_parameter("oT", [D, T], F32, isOutput=True)
    lv_d = nc.declare_dram_parameter("lv", [1, T], F32, isOutput=True)

    with TileContext(nc) as tc:
        with (
            tc.tile_pool(name="sb", bufs=1) as sb,
            tc.tile_pool(name="stp", bufs=1, space="PSUM") as stp,
            tc.tile_pool(name="pp", bufs=2, space="PSUM") as pp,
            tc.tile_pool(name="op", bufs=1, space="PSUM") as op,
            tc.tile_pool(name="lp", bufs=1, space="PSUM") as lp,
            tc.tile_pool(name="osb", bufs=4) as osb,
        ):
            # ---- input DMAs, critical-path order ----
            cb16 = sb.tile([D, 768], BF16)
            nc.sync.dma_start(out=cb16, in_=cb16_d[:, :])
            xin = sb.tile([D, 5120], BF16)
            nc.sync.dma_start(out=xin[:, 0:1024], in_=xin_d[:, 0:1024])
            cf32 = sb.tile([D, 132], F32R)
            nc.sync.dma_start(out=cf32, in_=cf32_d[:, :])
            nc.sync.dma_start(out=xin[:, 1024:2048], in_=xin_d[:, 1024:2048])
            ones8 = sb.tile([D, 2 * D], F8)
            nc.sync.dma_start(out=ones8, in_=ones8_d[:, :])
            nc.sync.dma_start(out=xin[:, 4096:5120], in_=xin_d[:, 4096:5120])
            nc.sync.dma_start(out=xin[:, 2048:3072], in_=xin_d[:, 2048:3072])
            nc.sync.dma_start(out=xin[:, 3072:4096], in_=xin_d[:, 3072:4096])

            wq = cb16[:, 0:128]
            wk = cb16[:, 128:256]
            wv = cb16[:, 256:384]
            msk = cb16[:, 384:640]     # [0:128) all NEG | [128:256) staircase
            ident = cb16[:, 640:768]
            bq = cf32[:, 0:1].bitcast(F32)
            sh4 = cf32[:, 1:2].bitcast(F32)  # -4.0 exp bias
            # l-matmul stationaries: 128-wide all-ones (the ISA rejects
            # narrow DoubleRow ldweights; a full-width stationary costs the
            # same moving columns and makes every PSUM row a copy of l)
            ones32 = cf32[:, 4:132]

            qt = sb.tile([D, T], F32R)
            kt = sb.tile([D, KV_TILES * 128], F32R)
            vsb32 = sb.tile([D, 2048], F32R)
            vsb8 = sb.tile([D, KV_TILES * 128], F8)
            lvs = sb.tile([1, T], F32)

            # ---- projection stages (pipelined into the attention stream).
            # Copies alternate DVE / GPSIMD so ACT stays exp-only. ----
            def xcol(t):  # xin column of kv tile t
                return t * 128 if t < 16 else 4096 + (t - 16) * 128

            def v_stage(g):          # V tiles 4g..4g+3 -> [kvrow, e]
                ps = pp.tile([D, CHUNK], F32, tag="pp", name=f"psv{g}")
                for jj in range(4):
                    t = 4 * g + jj
                    nc.tensor.matmul(
                        ps[:, jj * 128:(jj + 1) * 128],
                        xin[:, xcol(t):xcol(t) + 128], wv,
                        start=True, stop=True, skip_group_check=True,
                    )
                # GPSIMD cannot touch PSUM: DVE drains the bank, GPSIMD does
                # the off-critical-path SBUF->SBUF fp8 conversion.
                sl = slice(g * CHUNK, (g + 1) * CHUNK)
                if g < 4:
                    nc.vector.tensor_copy(vsb32[:, sl], ps)
                    nc.gpsimd.tensor_copy(vsb8[:, sl], vsb32[:, sl])
                else:
                    nc.vector.tensor_copy(vsb8[:, sl], ps)

            def k_stage(j):          # K^T chunk j
                ps = pp.tile([D, CHUNK], F32, tag="pp", name=f"psk{j}")
                src = xin[:, j * CHUNK:(j + 1) * CHUNK] if j < 4 else \
                    xin[:, 4096 + (j - 4) * CHUNK:4096 + (j - 3) * CHUNK]
                nc.tensor.matmul(ps, wk, src,
                                 start=True, stop=True, skip_group_check=True)
                nc.vector.tensor_copy(kt[:, j * CHUNK:(j + 1) * CHUNK], ps)

            def q_stage(c):          # Q^T chunk c (scaled, biased)
                ps = pp.tile([D, CHUNK], F32, tag="pp", name=f"psq{c}")
                nc.tensor.matmul(ps, wq, xin[:, c * CHUNK:(c + 1) * CHUNK],
                                 start=True, stop=True, skip_group_check=True)
                nc.vector.tensor_scalar_add(
                    qt[:, c * CHUNK:(c + 1) * CHUNK], ps, bq)

            # two persistent score buffers, manually rotated; zeroed once
            # because the merged diag exp reads a small inter-tile gap that
            # is otherwise uninitialized on first use (stale-but-finite
            # afterwards, never consumed)
            sts = [stp.tile([D, 2 * CHUNK], F32, name=f"st{i}")
                   for i in range(2)]
            for stz in sts:
                nc.vector.memset(stz, 0.0)
            nst = [0]

            # prologue: stage 0 + most of stage 1 before attention begins.
            # q/k first so chunk 0's first ST isn't gated on late copies.
            q_stage(0), k_stage(0), v_stage(0), v_stage(1), k_stage(1)
            thunkq = [lambda: q_stage(1)]
            stage_thunks = {
                2: [lambda: v_stage(2), lambda: k_stage(2), lambda: q_stage(2)],
                3: [lambda: v_stage(3), lambda: k_stage(3), lambda: q_stage(3)],
                4: [lambda: v_stage(4), lambda: v_stage(5), lambda: k_stage(4),
                    lambda: k_stage(5), lambda: q_stage(4)],
                5: [lambda: q_stage(5)],
                6: [lambda: q_stage(6)],
                7: [lambda: q_stage(7)],
            }

            # ---- attention: 8 chunks; units are kv-tile pairs.
            # Tri chunks: 2 diagonal (masked, f32r) pairs first, then full
            # pairs (fp8 DoubleRow) descending. Skew-1 software pipeline:
            # AV+l of unit u are emitted after ST/exp of unit u+1. ----
            units = []
            for c in range(NCHUNK):
                if c < 4:
                    pairs = [((4 * c, 4 * c + 1), (0, 128), False),
                             ((4 * c + 2, 4 * c + 3), (256, 256), False)]
                    for t0 in range(4 * c - 2, -1, -2):
                        pairs.append(((t0, t0 + 1), (0, 0), True))
                else:
                    pairs = [((16 + 2 * i, 17 + 2 * i), (0, 0), True)
                             for i in range(4)]
                for pi, (pr, los, is_fp8) in enumerate(pairs):
                    units.append((c, pr, los, is_fp8, pi == 0,
                                  pi == len(pairs) - 1))

            pts8 = [sb.tile([D, 2 * CHUNK], F8, name=f"pt8_{i}")
                    for i in range(3)]
            pts32 = [sb.tile([D, 2 * CHUNK], F32R, name=f"pt32_{i}")
                     for i in range(3)]
            n8 = [0]
            n32 = [0]
            acc = {}                # chunk -> (po, pl)
            pending = None
            epiq = []

            def emit_epilogue():
                c, po, pl = epiq.pop(0)
                qsl = slice(c * CHUNK, (c + 1) * CHUNK)
                ob = osb.tile([D, CHUNK], F32, tag="ob", name="ob")
                nc.vector.tensor_copy(ob, po)
                nc.sync.dma_start(out=ot_d[:, qsl], in_=ob)
                nc.vector.tensor_copy(lvs[:, qsl], pl[0:1, :])

            def emit_av(pend):
                c, pr, los, is_fp8, is_first, is_last, pt = pend
                if c not in acc:
                    acc[c] = (
                        op.tile([D, CHUNK], F32, tag="po", name="po"),
                        lp.tile([D, CHUNK], F32, tag="pl", name="pl"),
                    )
                po, pl = acc[c]
                if is_fp8:
                    v3 = vsb8[:, pr[0] * 128:(pr[0] + 2) * 128].rearrange(
                        "p (k f) -> p k f", k=2)
                    p3 = pt.rearrange("p (k f) -> p k f", k=2)
                    o3 = ones8.rearrange("p (k f) -> p k f", k=2)
                    nc.tensor.matmul(po, v3, p3, start=is_first, stop=is_last,
                                     perf_mode=DR, skip_group_check=True)
                    nc.tensor.matmul(pl, o3, p3, start=is_first,
                                     stop=is_last, perf_mode=DR,
                                     skip_group_check=True)
                else:
                    for i, t in enumerate(pr):
                        lo = los[i]
                        ptc = pt[:, i * CHUNK + lo:(i + 1) * CHUNK]
                        st_f = is_first and i == 0
                        sp_f = is_last and i == 1
                        nc.tensor.matmul(
                            po[:, lo:], vsb32[:, t * 128:(t + 1) * 128], ptc,
                            start=st_f, stop=sp_f, skip_group_check=True)
                        nc.tensor.matmul(
                            pl[:, lo:], ones32, ptc,
                            start=st_f, stop=sp_f, skip_group_check=True)
                if is_last:
                    epiq.append((c, po, pl))
                    del acc[c]

            for c, pr, los, is_fp8, is_first, is_last in units:
                if is_first and (c + 2) in stage_thunks:
                    thunkq.extend(stage_thunks[c + 2])
                if epiq:
                    emit_epilogue()
                for _ in range(2):
                    if thunkq:
                        thunkq.pop(0)()
                st = sts[nst[0] % 2]
                nst[0] += 1
                for i, t in enumerate(pr):
                    lo = los[i]
                    nc.tensor.matmul(
                        st[:, i * CHUNK + lo:(i + 1) * CHUNK],
                        kt[:, t * 128:(t + 1) * 128],
                        qt[:, c * CHUNK + lo:(c + 1) * CHUNK],
                        start=True, stop=True, skip_group_check=True,
                    )
                    if not is_fp8:
                        m = t - 4 * c
                        if m < 3:   # 128-wide staircase band
                            nc.tensor.matmul(
                                st[:, i * CHUNK + 128 * m:
                                   i * CHUNK + 128 * m + 128],
                                ident, msk[:, 128:256],
                                start=False, stop=True, skip_group_check=True)
                        else:       # 256-wide: fully-masked prefix + band
                            nc.tensor.matmul(
                                st[:, i * CHUNK + 256:(i + 1) * CHUNK],
                                ident, msk[:, 0:256],
                                start=False, stop=True, skip_group_check=True)
                if is_fp8:
                    pt = pts8[n8[0] % 3]
                    n8[0] += 1
                else:
                    pt = pts32[n32[0] % 3]
                    n32[0] += 1
                # one exp per pair; the inter-tile gap region (columns
                # [CHUNK, CHUNK+los[1]) when los[1] > 0) holds stale PSUM,
                # is exp'd harmlessly, and is never read downstream.
                nc.scalar.activation(
                    pt[:, los[0]:], st[:, los[0]:],
                    mybir.ActivationFunctionType.Exp, bias=sh4)
                prev = pending
                pending = (c, pr, los, is_fp8, is_first, is_last, pt)
                if prev is not None:
                    emit_av(prev)
            emit_av(pending)
            while epiq:
                emit_epilogue()
            nc.sync.dma_start(out=lv_d[:, :], in_=lvs)

    if legalize:
        _legalize_multiwaits(nc)
    nc.finalize()
    return nc


def _legalize_multiwaits(nc):
    """Hardware instruction structs in this walrus build accept at most ONE
    sync wait. Move all but the last wait onto single-wait same-engine NoOps
    inserted right before the instruction (engines execute in order)."""
    for fn in nc.m.functions:
        for blk in fn.blocks:
            insts = blk.instructions
            out = []
            for inst in insts:
                si = inst.sync_info
                if si is not None and si.on_wait and len(si.on_wait) >= 2:
                    waits = list(si.on_wait)
                    for w in waits[:-1]:
                        out.append(mybir.InstNoOp(
                            name=nc.get_next_instruction_name(),
                            engine=inst.engine,
                            bass_nofuse=True,
                            sync_info=mybir.SyncInfo(
                                on_wait=[w], on_update=[]),
                        ))
                    inst.sync_info = mybir.SyncInfo(
                        on_wait=[waits[-1]],
                        on_update=list(si.on_update or []))
                out.append(inst)
            insts[:] = out


_NC_CACHE = {}


def get_nc(legalize=True):
    key = ("nc", legalize)
    if key not in _NC_CACHE:
        _NC_CACHE[key] = build_nc(legalize)
    return _NC_CACHE[key]


def make_core_inputs(x, Wq, bq, Wk, bk, Wv, bv):
    """Per-core input maps (host-side sharding). bk dropped (softmax
    invariance); bv applied on the host."""
    import ml_dtypes

    s = 1.0 / math.sqrt(D)
    wq16 = (np.asarray(Wq, np.float32) * s).astype(ml_dtypes.bfloat16)
    wk16 = np.asarray(Wk, np.float32).astype(ml_dtypes.bfloat16)
    wv16 = np.asarray(Wv, np.float32).astype(ml_dtypes.bfloat16)

    # msk: cols [0:128) all NEG; [128:256) staircase 0 if j >= k else NEG
    kk = np.arange(128)[:, None]
    jj = np.arange(128)[None, :]
    stair = np.where(jj >= kk, 0.0, NEG).astype(np.float32)
    mskf = np.concatenate([np.full((D, 128), NEG, np.float32), stair], axis=1)
    identf = np.eye(D, dtype=np.float32)
    cb16 = np.concatenate(
        [wq16, wk16, wv16,
         mskf.astype(ml_dtypes.bfloat16), identf.astype(ml_dtypes.bfloat16)],
        axis=1)  # [D, 768] bf16

    cf32 = np.zeros((D, 132), np.float32)
    cf32[:, 0] = np.asarray(bq, np.float32) * s
    cf32[:, 1] = -SHIFT
    cf32[:, 4:132] = 1.0     # f32r all-ones stationary for diag l-matmuls
    ones8 = np.ones((D, 2 * D), ml_dtypes.float8_e4m3)

    x = np.asarray(x, dtype=np.float32)
    in_maps = []
    for core in range(8):
        b, h = core // 2, core % 2
        xb = x[b]
        tri = xb[h * HALF:(h + 1) * HALF]          # [2048, 128]
        rect_q = xb[HALF:]                         # [2048, 128]
        rect_kv = xb[h * 1024:(h + 1) * 1024]      # [1024, 128]
        xin = np.ascontiguousarray(
            np.concatenate([tri, rect_q, rect_kv], axis=0).T
        ).astype(ml_dtypes.bfloat16)               # [128, 5120]
        in_maps.append({"xin": xin, "cb16": cb16, "cf32": cf32,
                        "ones8": ones8})
    return in_maps


def merge_outputs(results, bv):
    """Gather per-core (oT, lv) into the full [B, T, D] output. The -4
    score shift scales o and l identically, so it cancels in o/l."""
    bv = np.asarray(bv, dtype=np.float32)
    out = np.empty((B, T, D), np.float32)
    for b in range(B):
        lo, hi = results[2 * b], results[2 * b + 1]
        lo_lv = np.asarray(lo["lv"]).reshape(NCHUNK, CHUNK)
        hi_lv = np.asarray(hi["lv"]).reshape(NCHUNK, CHUNK)
        O = np.zeros((T, D), np.float64)
        L = np.zeros(T, np.float64)
        O[:HALF] += lo["oT"][:, :HALF].T
        L[:HALF] += lo_lv[0:4].ravel()
        O[HALF:] += hi["oT"][:, :HALF].T
        L[HALF:] += hi_lv[0:4].ravel()
        O[HALF:] += lo["oT"][:, HALF:].T
        L[HALF:] += lo_lv[4:8].ravel()
        O[HALF:] += hi["oT"][:, HALF:].T
        L[HALF:] += hi_lv[4:8].ravel()
        out[b] = (O / L[:, None]).astype(np.float32) + bv
    return out


def run_per_core(nc, in_maps, threads=True):
    """Run the same single-core program on each NeuronCore with its own
    inputs (per-core dispatch; the cores share no collectives)."""
    import jax
    from concourse import bass2jax

    devices = jax.devices()[:len(in_maps)]

    def one(i):
        with jax.default_device(devices[i]):
            return bass2jax.run_bass_via_pjrt(nc, [in_maps[i]], n_cores=1)[0]

    if threads:
        from concurrent.futures import ThreadPoolExecutor
        first = one(0)
        with ThreadPoolExecutor(max_workers=7) as ex:
            rest = list(ex.map(one, range(1, len(in_maps))))
        return [first] + rest
    return [one(i) for i in range(len(in_maps))]


def kernel(x, Wq, bq, Wk, bk, Wv, bv, _trace=False):
    from concourse.bass_utils import axon_active, run_bass_kernel_spmd

    nc = get_nc()
    in_maps = make_core_inputs(x, Wq, bq, Wk, bk, Wv, bv)
    if axon_active():
        results = run_per_core(nc, in_maps)
    else:
        res = run_bass_kernel_spmd(nc, in_maps, list(range(8)), trace=_trace)
        kernel.last_result = res
        results = res.results
    out = merge_outputs(results, bv)
    return out


# revision 34
# speedup vs baseline: 1.3977x; 1.0188x over previous
"""Trainium2 Bass kernel: single-head causal attention (v2).

Problem: x[4,4096,128]; Q/K/V linear projections (W [in,out] layout, +bias);
scores = QK^T/sqrt(128) with causal mask; softmax; out = P @ V.

Sharding (8 cores = 4 batches x 2), SPMD (all cores run one program):
  core (b, h):
    triangle: queries q in [2048h, 2048h+2048) of batch b attending causally
        to kv rows in the same range.
    rectangle: queries q in [2048, 4096) attending to kv [1024h, 1024h+1024)
        (fully valid, no mask).
  Union over both cores of a batch covers the full causal set exactly once.

Softmax without max subtraction but with a constant shift: every score gets
-4.0 before exp (softmax shift-invariance keeps O/l exact), which brings
P = exp(s-4) into fp8 e4m3 range (max ~6 << 240). The cross-core merge stays
linear: host sums unnormalized o and denominators l, then divides.

Numerics by stage (validated to 6.8e-3 rel err vs the fp32 reference):
  - x and W DMA'd in bf16; projections are bf16 matmuls (1 cycle/row always,
    no fp32r narrow-output penalty), PSUM fp32.
  - QT/KT stored f32r; score matmuls (ST) run fp32r (full rate >= 256 wide).
  - Full (unmasked) kv-tile pairs: exp writes P~T in fp8 e4m3; AV and the
    denominator matmul run in DoubleRow perf mode (two 128-kv tiles per
    pass, 0.5 cycles/row = 4x the fp32r rate). V kept in fp8 (vsb8).
  - Diagonal (causally masked) pairs stay f32r end to end: short softmax
    rows have no error averaging, fp8 there fails the gate (5.8e-2).
    Tile m=3 is widened to a 256-col window (fully-masked prefix) so every
    diagonal ST/AV/l matmul is >= 256 wide (no fp32r 4x narrow penalty).
  - bk drops out of softmax; bq (pre-scaled) added during the Q PSUM->SBUF
    copy; bv added by the host after normalization.

Schedule: projections are software-pipelined INTO the attention chunk
stream (one or two projection matmuls between attention units) so the
scalar engine starts exp'ing ~3.5us in and never starves; the exp stream
(~36us on ACT) is the critical resource, with PE at ~33us underneath it.
Mask adds are bf16 identity-matmuls on the PE (no ACT/DVE time). PSUM->
SBUF copies and output staging are split between DVE and GPSIMD(Pool),
keeping ACT exp-only. PSUM banks: st 2x2 + proj 2x1 + po 1 + pl 1 = 8.

Device layouts (per core):
  xin  [128,5120] bf16  x^T cols: 0:4096 = (tri|rect_q), 4096:5120 = rect_kv
  cb16 [128,768]  bf16  wq|wk|wv|msk(256)|ident
  cf32 [128,4]    f32r  bq' | -4.0 | 1.0 (ones) | fp8-ones bit pattern
  QT [128,4096] f32r; KT [128,3072] f32r
  vsb32 [128,2048] f32r (V tiles 0-15, diag pairs)
  vsb8  [128,3072] fp8  (V tiles 0-23, full pairs)
  ST computed transposed: ST[k,q] = K Q^T in PSUM; exp(ST-4) -> P~T in
  fp8 (full pairs) or f32r (diag pairs); AV: oT[e,q] += V^T-mm-P~T;
  l[q] += ones-mm-P~T (PE is the only partition reducer).
Outputs: oT [128,4096] f32 (transposed, unnormalized), lv [1,4096]
(denominators, chunk-major). Host transposes, merges, normalizes, adds bv.
"""

import math
import sys

import numpy as np

sys.path.insert(0, "/opt/trn_rl_repo")

import concourse.bass as bass  # noqa: E402
import concourse.mybir as mybir  # noqa: E402
from concourse.tile import TileContext  # noqa: E402

B, T, D = 4, 4096, 128
HALF = T // 2          # 2048 queries per triangle
NCHUNK = 8             # 8 chunks of 512 query slots (4 tri + 4 rect)
CHUNK = 512
KV_TILES = 24          # 16 tri + 8 rect kv tiles of 128 rows
NEG = -99840.0         # additive mask value, exactly representable in bf16
SHIFT = 4.0            # score shift: exp(s - 4) keeps P in fp8 range

F32 = mybir.dt.float32
F32R = mybir.dt.float32r
BF16 = mybir.dt.bfloat16
F8 = mybir.dt.float8e4
DR = mybir.MatmulPerfMode.DoubleRow


def round_f32r(a):
    """Exact fp32 -> fp32r rounding (RNE to 11 mantissa bits)."""
    u = np.ascontiguousarray(a, np.float32).view(np.uint32)
    add = np.uint32(0x7FF) + ((u >> np.uint32(12)) & np.uint32(1))
    return ((u + add) & np.uint32(0xFFFFF000)).view(np.float32)


def build_nc(legalize=True):
    nc = bass.Bass()

    xin_d = nc.declare_dram_parameter("xin", [D, 5120], BF16, isOutput=False)
    cb16_d = nc.declare_dram_parameter("cb16", [D, 768], BF16, isOutput=False)
    cf32_d = nc.declare_dram_parameter("cf32", [D, 132], F32R, isOutput=False)
    ones8_d = nc.declare_dram_parameter("ones8", [D, 2 * D], F8, isOutput=False)

    ot_d = nc.declare_dram_parameter("oT", [D, T], F32, isOutput=True)
    lv_d = nc.declare_dram_parameter("lv", [1, T], F32, isOutput=True)

    with TileContext(nc) as tc:
        with (
            tc.tile_pool(name="sb", bufs=1) as sb,
            tc.tile_pool(name="stp", bufs=1, space="PSUM") as stp,
            tc.tile_pool(name="pp", bufs=2, space="PSUM") as pp,
            tc.tile_pool(name="op", bufs=1, space="PSUM") as op,
            tc.tile_pool(name="lp", bufs=1, space="PSUM") as lp,
            tc.tile_pool(name="osb", bufs=4) as osb,
        ):
            # ---- input DMAs, critical-path order ----
            cb16 = sb.tile([D, 768], BF16)
            nc.sync.dma_start(out=cb16, in_=cb16_d[:, :])
            xin = sb.tile([D, 5120], BF16)
            nc.sync.dma_start(out=xin[:, 0:1024], in_=xin_d[:, 0:1024])
            cf32 = sb.tile([D, 132], F32R)
            nc.sync.dma_start(out=cf32, in_=cf32_d[:, :])
            nc.sync.dma_start(out=xin[:, 1024:2048], in_=xin_d[:, 1024:2048])
            ones8 = sb.tile([D, 2 * D], F8)
            nc.sync.dma_start(out=ones8, in_=ones8_d[:, :])
            nc.sync.dma_start(out=xin[:, 4096:5120], in_=xin_d[:, 4096:5120])
            nc.sync.dma_start(out=xin[:, 2048:3072], in_=xin_d[:, 2048:3072])
            nc.sync.dma_start(out=xin[:, 3072:4096], in_=xin_d[:, 3072:4096])

            wq = cb16[:, 0:128]
            wk = cb16[:, 128:256]
            wv = cb16[:, 256:384]
            msk = cb16[:, 384:640]     # [0:128) all NEG | [128:256) staircase
            ident = cb16[:, 640:768]
            bq = cf32[:, 0:1].bitcast(F32)
            sh4 = cf32[:, 1:2].bitcast(F32)  # -4.0 exp bias
            # l-matmul stationaries: 128-wide all-ones (the ISA rejects
            # narrow DoubleRow ldweights; a full-width stationary costs the
            # same moving columns and makes every PSUM row a copy of l)
            ones32 = cf32[:, 4:132]

            qt = sb.tile([D, T], F32R)
            kt = sb.tile([D, KV_TILES * 128], F32R)
            vsb32 = sb.tile([D, CHUNK], F32R)   # V tiles 0-3 (chunk-0 diag)
            vsb8 = sb.tile([D, KV_TILES * 128], F8)
            lvs = sb.tile([1, T], F32)

            # ---- projection stages (pipelined into the attention stream).
            # Copies alternate DVE / GPSIMD so ACT stays exp-only. ----
            def xcol(t):  # xin column of kv tile t
                return t * 128 if t < 16 else 4096 + (t - 16) * 128

            def v_stage(g):          # V tiles 4g..4g+3 -> [kvrow, e]
                ps = pp.tile([D, CHUNK], F32, tag="pp", name=f"psv{g}")
                for jj in range(4):
                    t = 4 * g + jj
                    nc.tensor.matmul(
                        ps[:, jj * 128:(jj + 1) * 128],
                        xin[:, xcol(t):xcol(t) + 128], wv,
                        start=True, stop=True, skip_group_check=True,
                    )
                # GPSIMD cannot touch PSUM: DVE drains the bank, GPSIMD does
                # the off-critical-path SBUF->SBUF fp8 conversion.
                sl = slice(g * CHUNK, (g + 1) * CHUNK)
                if g == 0:
                    nc.vector.tensor_copy(vsb32, ps)
                    nc.gpsimd.tensor_copy(vsb8[:, sl], vsb32)
                else:
                    nc.vector.tensor_copy(vsb8[:, sl], ps)

            def k_stage(j):          # K^T chunk j
                ps = pp.tile([D, CHUNK], F32, tag="pp", name=f"psk{j}")
                src = xin[:, j * CHUNK:(j + 1) * CHUNK] if j < 4 else \
                    xin[:, 4096 + (j - 4) * CHUNK:4096 + (j - 3) * CHUNK]
                nc.tensor.matmul(ps, wk, src,
                                 start=True, stop=True, skip_group_check=True)
                nc.vector.tensor_copy(kt[:, j * CHUNK:(j + 1) * CHUNK], ps)

            def q_stage(c):          # Q^T chunk c (scaled, biased)
                ps = pp.tile([D, CHUNK], F32, tag="pp", name=f"psq{c}")
                nc.tensor.matmul(ps, wq, xin[:, c * CHUNK:(c + 1) * CHUNK],
                                 start=True, stop=True, skip_group_check=True)
                if c == 0:
                    # ACT is idle before the first exp; doing the chunk-0
                    # bias-copy there unblocks DVE for the k/v copies
                    nc.scalar.activation(
                        qt[:, 0:CHUNK], ps,
                        mybir.ActivationFunctionType.Identity, bias=bq)
                else:
                    nc.vector.tensor_scalar_add(
                        qt[:, c * CHUNK:(c + 1) * CHUNK], ps, bq)

            # two persistent score buffers, manually rotated. The merged
            # diag exp reads a small inter-tile gap; only chunk 0's first
            # use of each buffer sees it uninitialized (stale-but-finite
            # afterwards, never consumed) - zero exactly those windows.
            sts = [stp.tile([D, 2 * CHUNK], F32, name=f"st{i}")
                   for i in range(2)]
            nc.vector.memset(sts[0][:, CHUNK:CHUNK + 128], 0.0)
            nc.vector.memset(sts[1][:, CHUNK:CHUNK + 256], 0.0)
            nst = [0]

            # prologue: stage 0 + most of stage 1 before attention begins.
            # q/k first so chunk 0's first ST isn't gated on late copies.
            q_stage(0), k_stage(0), v_stage(0), v_stage(1), k_stage(1)
            thunkq = [lambda: q_stage(1)]
            stage_thunks = {
                2: [lambda: v_stage(2), lambda: k_stage(2), lambda: q_stage(2)],
                3: [lambda: v_stage(3), lambda: k_stage(3), lambda: q_stage(3)],
                4: [lambda: v_stage(4), lambda: v_stage(5), lambda: k_stage(4),
                    lambda: k_stage(5), lambda: q_stage(4)],
                5: [lambda: q_stage(5)],
                6: [lambda: q_stage(6)],
                7: [lambda: q_stage(7)],
            }

            # ---- attention: 8 chunks; units are kv-tile pairs.
            # Tri chunks: 2 diagonal (masked) pairs first, then full pairs
            # (fp8 DoubleRow) descending. Only chunk 0's diagonals (the
            # short softmax rows, no error averaging) stay f32r; chunks
            # 1-3 diagonals run fp8 DoubleRow with a uniform per-pair lo.
            # Skew-1 software pipeline: AV+l of unit u are emitted after
            # ST/exp of unit u+1. Kinds: 'f32' chunk-0 diag, 'f8d' fp8
            # diag (masked), 'f8' full. ----
            units = []
            for c in range(NCHUNK):
                if c == 0:
                    pairs = [((0, 1), (0, 128), "f32"),
                             ((2, 3), (256, 256), "f32")]
                elif c < 4:
                    pairs = [((4 * c, 4 * c + 1), (0, 0), "f8d"),
                             ((4 * c + 2, 4 * c + 3), (256, 256), "f8d")]
                    for t0 in range(4 * c - 2, -1, -2):
                        pairs.append(((t0, t0 + 1), (0, 0), "f8"))
                else:
                    pairs = [((16 + 2 * i, 17 + 2 * i), (0, 0), "f8")
                             for i in range(4)]
                for pi, (pr, los, kind) in enumerate(pairs):
                    units.append((c, pr, los, kind, pi == 0,
                                  pi == len(pairs) - 1))

            pts8 = [sb.tile([D, 2 * CHUNK], F8, name=f"pt8_{i}")
                    for i in range(3)]
            pts32 = [sb.tile([D, 2 * CHUNK], F32R, name=f"pt32_{i}")
                     for i in range(2)]
            n8 = [0]
            n32 = [0]
            acc = {}                # chunk -> (po, pl)
            pending = None
            epiq = []

            def emit_epilogue():
                c, po, pl = epiq.pop(0)
                qsl = slice(c * CHUNK, (c + 1) * CHUNK)
                ob = osb.tile([D, CHUNK], F32, tag="ob", name="ob")
                nc.vector.tensor_copy(ob, po)
                nc.sync.dma_start(out=ot_d[:, qsl], in_=ob)
                nc.vector.tensor_copy(lvs[:, qsl], pl[0:1, :])

            def emit_av(pend):
                c, pr, los, kind, is_first, is_last, pt = pend
                if c not in acc:
                    acc[c] = (
                        op.tile([D, CHUNK], F32, tag="po", name="po"),
                        lp.tile([D, CHUNK], F32, tag="pl", name="pl"),
                    )
                po, pl = acc[c]
                if kind != "f32":
                    lo = los[0]       # uniform per-pair lo for fp8 kinds
                    v3 = vsb8[:, pr[0] * 128:(pr[0] + 2) * 128].rearrange(
                        "p (k f) -> p k f", k=2)
                    p3 = pt.rearrange("p (k f) -> p k f", k=2)[:, :, lo:]
                    o3 = ones8.rearrange("p (k f) -> p k f", k=2)
                    nc.tensor.matmul(po[:, lo:], v3, p3, start=is_first,
                                     stop=is_last, perf_mode=DR,
                                     skip_group_check=True)
                    nc.tensor.matmul(pl[:, lo:], o3, p3, start=is_first,
                                     stop=is_last, perf_mode=DR,
                                     skip_group_check=True)
                else:
                    for i, t in enumerate(pr):
                        lo = los[i]
                        ptc = pt[:, i * CHUNK + lo:(i + 1) * CHUNK]
                        st_f = is_first and i == 0
                        sp_f = is_last and i == 1
                        nc.tensor.matmul(
                            po[:, lo:], vsb32[:, t * 128:(t + 1) * 128], ptc,
                            start=st_f, stop=sp_f, skip_group_check=True)
                        nc.tensor.matmul(
                            pl[:, lo:], ones32, ptc,
                            start=st_f, stop=sp_f, skip_group_check=True)
                if is_last:
                    epiq.append((c, po, pl))
                    del acc[c]

            for c, pr, los, kind, is_first, is_last in units:
                if is_first and (c + 2) in stage_thunks:
                    thunkq.extend(stage_thunks[c + 2])
                if epiq:
                    emit_epilogue()
                for _ in range(2):
                    if thunkq:
                        thunkq.pop(0)()
                st = sts[nst[0] % 2]
                nst[0] += 1
                for i, t in enumerate(pr):
                    lo = los[i]
                    nc.tensor.matmul(
                        st[:, i * CHUNK + lo:(i + 1) * CHUNK],
                        kt[:, t * 128:(t + 1) * 128],
                        qt[:, c * CHUNK + lo:(c + 1) * CHUNK],
                        start=True, stop=True, skip_group_check=True,
                    )
                    if kind != "f8":
                        # causal mask band over [lo, 128(m+1)): width w
                        # staircase tail, all-NEG before it. msk stores
                        # [allNEG(128) | staircase(128)]; slice the last
                        # w columns.
                        m = t - 4 * c
                        w = 128 * (m + 1) - lo
                        nc.tensor.matmul(
                            st[:, i * CHUNK + lo:i * CHUNK + lo + w],
                            ident, msk[:, 256 - w:256],
                            start=False, stop=True, skip_group_check=True)
                if kind == "f32":
                    pt = pts32[n32[0] % 2]
                    n32[0] += 1
                else:
                    pt = pts8[n8[0] % 3]
                    n8[0] += 1
                # one exp per pair; the inter-tile gap region (columns
                # [CHUNK, CHUNK+los[1]) when los[1] > los[0]) holds stale
                # PSUM, is exp'd harmlessly, and is never read downstream.
                nc.scalar.activation(
                    pt[:, los[0]:], st[:, los[0]:],
                    mybir.ActivationFunctionType.Exp, bias=sh4)
                prev = pending
                pending = (c, pr, los, kind, is_first, is_last, pt)
                if prev is not None:
                    emit_av(prev)
            emit_av(pending)
            while epiq:
                emit_epilogue()
            nc.sync.dma_start(out=lv_d[:, :], in_=lvs)

    if legalize:
        _legalize_multiwaits(nc)
    nc.finalize()
    return nc


def _legalize_multiwaits(nc):
    """Hardware instruction structs in this walrus build accept at most ONE
    sync wait. Move all but the last wait onto single-wait same-engine NoOps
    inserted right before the instruction (engines execute in order)."""
    for fn in nc.m.functions:
        for blk in fn.blocks:
            insts = blk.instructions
            out = []
            for inst in insts:
                si = inst.sync_info
                if si is not None and si.on_wait and len(si.on_wait) >= 2:
                    waits = list(si.on_wait)
                    for w in waits[:-1]:
                        out.append(mybir.InstNoOp(
                            name=nc.get_next_instruction_name(),
                            engine=inst.engine,
                            bass_nofuse=True,
                            sync_info=mybir.SyncInfo(
                                on_wait=[w], on_update=[]),
                        ))
                    inst.sync_info = mybir.SyncInfo(
                        on_wait=[waits[-1]],
                        on_update=list(si.on_update or []))
                out.append(inst)
            insts[:] = out


_NC_CACHE = {}


def get_nc(legalize=True):
    key = ("nc", legalize)
    if key not in _NC_CACHE:
        _NC_CACHE[key] = build_nc(legalize)
    return _NC_CACHE[key]


def make_core_inputs(x, Wq, bq, Wk, bk, Wv, bv):
    """Per-core input maps (host-side sharding). bk dropped (softmax
    invariance); bv applied on the host."""
    import ml_dtypes

    s = 1.0 / math.sqrt(D)
    wq16 = (np.asarray(Wq, np.float32) * s).astype(ml_dtypes.bfloat16)
    wk16 = np.asarray(Wk, np.float32).astype(ml_dtypes.bfloat16)
    wv16 = np.asarray(Wv, np.float32).astype(ml_dtypes.bfloat16)

    # msk: cols [0:128) all NEG; [128:256) staircase 0 if j >= k else NEG
    kk = np.arange(128)[:, None]
    jj = np.arange(128)[None, :]
    stair = np.where(jj >= kk, 0.0, NEG).astype(np.float32)
    mskf = np.concatenate([np.full((D, 128), NEG, np.float32), stair], axis=1)
    identf = np.eye(D, dtype=np.float32)
    cb16 = np.concatenate(
        [wq16, wk16, wv16,
         mskf.astype(ml_dtypes.bfloat16), identf.astype(ml_dtypes.bfloat16)],
        axis=1)  # [D, 768] bf16

    cf32 = np.zeros((D, 132), np.float32)
    cf32[:, 0] = np.asarray(bq, np.float32) * s
    cf32[:, 1] = -SHIFT
    cf32[:, 4:132] = 1.0     # f32r all-ones stationary for diag l-matmuls
    ones8 = np.ones((D, 2 * D), ml_dtypes.float8_e4m3)

    x = np.asarray(x, dtype=np.float32)
    in_maps = []
    for core in range(8):
        b, h = core // 2, core % 2
        xb = x[b]
        tri = xb[h * HALF:(h + 1) * HALF]          # [2048, 128]
        rect_q = xb[HALF:]                         # [2048, 128]
        rect_kv = xb[h * 1024:(h + 1) * 1024]      # [1024, 128]
        xin = np.ascontiguousarray(
            np.concatenate([tri, rect_q, rect_kv], axis=0).T
        ).astype(ml_dtypes.bfloat16)               # [128, 5120]
        in_maps.append({"xin": xin, "cb16": cb16, "cf32": cf32,
                        "ones8": ones8})
    return in_maps


def merge_outputs(results, bv):
    """Gather per-core (oT, lv) into the full [B, T, D] output. The -4
    score shift scales o and l identically, so it cancels in o/l."""
    bv = np.asarray(bv, dtype=np.float32)
    out = np.empty((B, T, D), np.float32)
    for b in range(B):
        lo, hi = results[2 * b], results[2 * b + 1]
        lo_lv = np.asarray(lo["lv"]).reshape(NCHUNK, CHUNK)
        hi_lv = np.asarray(hi["lv"]).reshape(NCHUNK, CHUNK)
        O = np.zeros((T, D), np.float64)
        L = np.zeros(T, np.float64)
        O[:HALF] += lo["oT"][:, :HALF].T
        L[:HALF] += lo_lv[0:4].ravel()
        O[HALF:] += hi["oT"][:, :HALF].T
        L[HALF:] += hi_lv[0:4].ravel()
        O[HALF:] += lo["oT"][:, HALF:].T
        L[HALF:] += lo_lv[4:8].ravel()
        O[HALF:] += hi["oT"][:, HALF:].T
        L[HALF:] += hi_lv[4:8].ravel()
        out[b] = (O / L[:, None]).astype(np.float32) + bv
    return out


def run_per_core(nc, in_maps, threads=True):
    """Run the same single-core program on each NeuronCore with its own
    inputs (per-core dispatch; the cores share no collectives)."""
    import jax
    from concourse import bass2jax

    devices = jax.devices()[:len(in_maps)]

    def one(i):
        with jax.default_device(devices[i]):
            return bass2jax.run_bass_via_pjrt(nc, [in_maps[i]], n_cores=1)[0]

    if threads:
        from concurrent.futures import ThreadPoolExecutor
        first = one(0)
        with ThreadPoolExecutor(max_workers=7) as ex:
            rest = list(ex.map(one, range(1, len(in_maps))))
        return [first] + rest
    return [one(i) for i in range(len(in_maps))]


def kernel(x, Wq, bq, Wk, bk, Wv, bv, _trace=False):
    from concourse.bass_utils import axon_active, run_bass_kernel_spmd

    nc = get_nc()
    in_maps = make_core_inputs(x, Wq, bq, Wk, bk, Wv, bv)
    if axon_active():
        results = run_per_core(nc, in_maps)
    else:
        res = run_bass_kernel_spmd(nc, in_maps, list(range(8)), trace=_trace)
        kernel.last_result = res
        results = res.results
    out = merge_outputs(results, bv)
    return out


# revision 41
# speedup vs baseline: 1.4108x; 1.0094x over previous
"""Trainium2 Bass kernel: single-head causal attention (v2).

Problem: x[4,4096,128]; Q/K/V linear projections (W [in,out] layout, +bias);
scores = QK^T/sqrt(128) with causal mask; softmax; out = P @ V.

Sharding (8 cores = 4 batches x 2), SPMD (all cores run one program):
  core (b, h):
    triangle: queries q in [2048h, 2048h+2048) of batch b attending causally
        to kv rows in the same range.
    rectangle: queries q in [2048, 4096) attending to kv [1024h, 1024h+1024)
        (fully valid, no mask).
  Union over both cores of a batch covers the full causal set exactly once.

Softmax without max subtraction but with a constant shift: every score gets
-4.0 before exp (softmax shift-invariance keeps O/l exact), which brings
P = exp(s-4) into fp8 e4m3 range (max ~6 << 240). The cross-core merge stays
linear: host sums unnormalized o and denominators l, then divides.

Numerics by stage (validated to 6.8e-3 rel err vs the fp32 reference):
  - x and W DMA'd in bf16; projections are bf16 matmuls (1 cycle/row always,
    no fp32r narrow-output penalty), PSUM fp32.
  - QT/KT stored f32r; score matmuls (ST) run fp32r (full rate >= 256 wide).
  - Full (unmasked) kv-tile pairs: exp writes P~T in fp8 e4m3; AV and the
    denominator matmul run in DoubleRow perf mode (two 128-kv tiles per
    pass, 0.5 cycles/row = 4x the fp32r rate). V kept in fp8 (vsb8).
  - Diagonal (causally masked) pairs stay f32r end to end: short softmax
    rows have no error averaging, fp8 there fails the gate (5.8e-2).
    Tile m=3 is widened to a 256-col window (fully-masked prefix) so every
    diagonal ST/AV/l matmul is >= 256 wide (no fp32r 4x narrow penalty).
  - bk drops out of softmax; bq (pre-scaled) added during the Q PSUM->SBUF
    copy; bv added by the host after normalization.

Schedule: projections are software-pipelined INTO the attention chunk
stream (one or two projection matmuls between attention units) so the
scalar engine starts exp'ing ~3.5us in and never starves; the exp stream
(~36us on ACT) is the critical resource, with PE at ~33us underneath it.
Mask adds are bf16 identity-matmuls on the PE (no ACT/DVE time). PSUM->
SBUF copies and output staging are split between DVE and GPSIMD(Pool),
keeping ACT exp-only. PSUM banks: st 2x2 + proj 2x1 + po 1 + pl 1 = 8.

Device layouts (per core):
  xin  [128,5120] bf16  x^T cols: 0:4096 = (tri|rect_q), 4096:5120 = rect_kv
  cb16 [128,768]  bf16  wq|wk|wv|msk(256)|ident
  cf32 [128,4]    f32r  bq' | -4.0 | 1.0 (ones) | fp8-ones bit pattern
  QT [128,4096] f32r; KT [128,3072] f32r
  vsb32 [128,2048] f32r (V tiles 0-15, diag pairs)
  vsb8  [128,3072] fp8  (V tiles 0-23, full pairs)
  ST computed transposed: ST[k,q] = K Q^T in PSUM; exp(ST-4) -> P~T in
  fp8 (full pairs) or f32r (diag pairs); AV: oT[e,q] += V^T-mm-P~T;
  l[q] += ones-mm-P~T (PE is the only partition reducer).
Outputs: oT [128,4096] f32 (transposed, unnormalized), lv [1,4096]
(denominators, chunk-major). Host transposes, merges, normalizes, adds bv.
"""

import math
import sys

import numpy as np

sys.path.insert(0, "/opt/trn_rl_repo")

import concourse.bass as bass  # noqa: E402
import concourse.mybir as mybir  # noqa: E402
from concourse.tile import TileContext  # noqa: E402

B, T, D = 4, 4096, 128
HALF = T // 2          # 2048 queries per triangle
NCHUNK = 8             # 8 chunks of 512 query slots (4 tri + 4 rect)
CHUNK = 512
KV_TILES = 24          # 16 tri + 8 rect kv tiles of 128 rows
NEG = -99840.0         # additive mask value, exactly representable in bf16
SHIFT = 4.0            # score shift: exp(s - 4) keeps P in fp8 range

F32 = mybir.dt.float32
F32R = mybir.dt.float32r
BF16 = mybir.dt.bfloat16
F8 = mybir.dt.float8e4
DR = mybir.MatmulPerfMode.DoubleRow


def round_f32r(a):
    """Exact fp32 -> fp32r rounding (RNE to 11 mantissa bits)."""
    u = np.ascontiguousarray(a, np.float32).view(np.uint32)
    add = np.uint32(0x7FF) + ((u >> np.uint32(12)) & np.uint32(1))
    return ((u + add) & np.uint32(0xFFFFF000)).view(np.float32)


def build_nc(legalize=True):
    nc = bass.Bass()

    xin_d = nc.declare_dram_parameter("xin", [D, 5120], BF16, isOutput=False)
    cb16_d = nc.declare_dram_parameter("cb16", [D, 768], BF16, isOutput=False)
    cf32_d = nc.declare_dram_parameter("cf32", [D, 132], F32R, isOutput=False)
    ones8_d = nc.declare_dram_parameter("ones8", [D, 2 * D], F8, isOutput=False)

    ot_d = nc.declare_dram_parameter("oT", [D, T], F32, isOutput=True)
    lv_d = nc.declare_dram_parameter("lv", [1, T], F32, isOutput=True)

    with TileContext(nc) as tc:
        with (
            tc.tile_pool(name="sb", bufs=1) as sb,
            tc.tile_pool(name="stp", bufs=1, space="PSUM") as stp,
            tc.tile_pool(name="pp", bufs=2, space="PSUM") as pp,
            tc.tile_pool(name="op", bufs=1, space="PSUM") as op,
            tc.tile_pool(name="lp", bufs=1, space="PSUM") as lp,
            tc.tile_pool(name="osb", bufs=4) as osb,
        ):
            # ---- input DMAs, critical-path order: stage 0 (weights + xin
            # cols 0:512 + consts) lands first so PE starts ~2.6us in ----
            cb16 = sb.tile([D, 768], BF16)
            nc.sync.dma_start(out=cb16, in_=cb16_d[:, :])
            xin = sb.tile([D, 5120], BF16)
            nc.sync.dma_start(out=xin[:, 0:512], in_=xin_d[:, 0:512])
            cf32 = sb.tile([D, 132], F32R)
            nc.sync.dma_start(out=cf32, in_=cf32_d[:, :])
            nc.sync.dma_start(out=xin[:, 512:1024], in_=xin_d[:, 512:1024])
            nc.sync.dma_start(out=xin[:, 1024:2048], in_=xin_d[:, 1024:2048])
            ones8 = sb.tile([D, 2 * D], F8)
            nc.sync.dma_start(out=ones8, in_=ones8_d[:, :])
            nc.sync.dma_start(out=xin[:, 4096:5120], in_=xin_d[:, 4096:5120])
            nc.sync.dma_start(out=xin[:, 2048:3072], in_=xin_d[:, 2048:3072])
            nc.sync.dma_start(out=xin[:, 3072:4096], in_=xin_d[:, 3072:4096])

            wq = cb16[:, 0:128]
            wk = cb16[:, 128:256]
            wv = cb16[:, 256:384]
            msk = cb16[:, 384:640]     # [0:128) all NEG | [128:256) staircase
            ident = cb16[:, 640:768]
            bq = cf32[:, 0:1].bitcast(F32)
            sh4 = cf32[:, 1:2].bitcast(F32)  # -4.0 exp bias
            # l-matmul stationaries: 128-wide all-ones (the ISA rejects
            # narrow DoubleRow ldweights; a full-width stationary costs the
            # same moving columns and makes every PSUM row a copy of l)
            ones32 = cf32[:, 4:132]

            qt = sb.tile([D, T], F32R)
            kt = sb.tile([D, KV_TILES * 128], F32R)
            vsb32 = sb.tile([D, CHUNK], F32R)   # V tiles 0-3 (chunk-0 diag)
            vsb8 = sb.tile([D, KV_TILES * 128], F8)
            lvs = sb.tile([1, T], F32)

            # ---- projection stages (pipelined into the attention stream).
            # Copies alternate DVE / GPSIMD so ACT stays exp-only. ----
            def xcol(t):  # xin column of kv tile t
                return t * 128 if t < 16 else 4096 + (t - 16) * 128

            def v_stage(g):          # V tiles 4g..4g+3 -> [kvrow, e]
                ps = pp.tile([D, CHUNK], F32, tag="pp", name=f"psv{g}")
                for jj in range(4):
                    t = 4 * g + jj
                    nc.tensor.matmul(
                        ps[:, jj * 128:(jj + 1) * 128],
                        xin[:, xcol(t):xcol(t) + 128], wv,
                        start=True, stop=True, skip_group_check=True,
                    )
                # GPSIMD cannot touch PSUM: DVE drains the bank, GPSIMD does
                # the off-critical-path SBUF->SBUF fp8 conversion.
                sl = slice(g * CHUNK, (g + 1) * CHUNK)
                if g == 0:
                    nc.vector.tensor_copy(vsb32, ps)
                    nc.gpsimd.tensor_copy(vsb8[:, sl], vsb32)
                else:
                    nc.vector.tensor_copy(vsb8[:, sl], ps)

            def k_stage(j):          # K^T chunk j
                ps = pp.tile([D, CHUNK], F32, tag="pp", name=f"psk{j}")
                src = xin[:, j * CHUNK:(j + 1) * CHUNK] if j < 4 else \
                    xin[:, 4096 + (j - 4) * CHUNK:4096 + (j - 3) * CHUNK]
                nc.tensor.matmul(ps, wk, src,
                                 start=True, stop=True, skip_group_check=True)
                nc.vector.tensor_copy(kt[:, j * CHUNK:(j + 1) * CHUNK], ps)

            def q_stage(c):          # Q^T chunk c (scaled, biased)
                ps = pp.tile([D, CHUNK], F32, tag="pp", name=f"psq{c}")
                nc.tensor.matmul(ps, wq, xin[:, c * CHUNK:(c + 1) * CHUNK],
                                 start=True, stop=True, skip_group_check=True)
                if c <= 1:
                    # ACT has idle slots before/between the first exps;
                    # early bias-copies there unblock DVE for the k/v copies
                    nc.scalar.activation(
                        qt[:, c * CHUNK:(c + 1) * CHUNK], ps,
                        mybir.ActivationFunctionType.Identity, bias=bq)
                else:
                    nc.vector.tensor_scalar_add(
                        qt[:, c * CHUNK:(c + 1) * CHUNK], ps, bq)

            # two persistent score buffers, manually rotated. The merged
            # diag exp reads a small inter-tile gap; only chunk 0's first
            # use of each buffer sees it uninitialized (stale-but-finite
            # afterwards, never consumed) - zero exactly those windows.
            sts = [stp.tile([D, 2 * CHUNK], F32, name=f"st{i}")
                   for i in range(2)]
            nc.vector.memset(sts[0][:, CHUNK:CHUNK + 128], 0.0)
            nc.vector.memset(sts[1][:, CHUNK:CHUNK + 256], 0.0)
            nst = [0]

            # prologue: stage 0 + most of stage 1 before attention begins.
            # q/k first so chunk 0's first ST isn't gated on late copies.
            q_stage(0), k_stage(0), v_stage(0), k_stage(1), v_stage(1)
            thunkq = [lambda: q_stage(1)]
            stage_thunks = {
                2: [lambda: v_stage(2), lambda: k_stage(2), lambda: q_stage(2)],
                3: [lambda: v_stage(3), lambda: k_stage(3), lambda: q_stage(3)],
                4: [lambda: v_stage(4), lambda: v_stage(5), lambda: k_stage(4),
                    lambda: k_stage(5), lambda: q_stage(4)],
                5: [lambda: q_stage(5)],
                6: [lambda: q_stage(6)],
                7: [lambda: q_stage(7)],
            }

            # ---- attention: 8 chunks; units are kv-tile pairs.
            # Tri chunks: 2 diagonal (masked) pairs first, then full pairs
            # (fp8 DoubleRow) descending. Only chunk 0's diagonals (the
            # short softmax rows, no error averaging) stay f32r; chunks
            # 1-3 diagonals run fp8 DoubleRow with a uniform per-pair lo.
            # Skew-1 software pipeline: AV+l of unit u are emitted after
            # ST/exp of unit u+1. Kinds: 'f32' chunk-0 diag, 'f8d' fp8
            # diag (masked), 'f8' full. ----
            units = []
            for c in range(NCHUNK):
                if c == 0:
                    pairs = [((0, 1), (0, 128), "f32"),
                             ((2, 3), (256, 256), "f32")]
                elif c < 4:
                    pairs = [((4 * c, 4 * c + 1), (0, 0), "f8d"),
                             ((4 * c + 2, 4 * c + 3), (256, 256), "f8d")]
                    for t0 in range(4 * c - 2, -1, -2):
                        pairs.append(((t0, t0 + 1), (0, 0), "f8"))
                else:
                    pairs = [((16 + 2 * i, 17 + 2 * i), (0, 0), "f8")
                             for i in range(4)]
                for pi, (pr, los, kind) in enumerate(pairs):
                    units.append((c, pr, los, kind, pi == 0,
                                  pi == len(pairs) - 1))

            pts8 = [sb.tile([D, 2 * CHUNK], F8, name=f"pt8_{i}")
                    for i in range(3)]
            pts32 = [sb.tile([D, 2 * CHUNK], F32R, name=f"pt32_{i}")
                     for i in range(2)]
            n8 = [0]
            n32 = [0]
            acc = {}                # chunk -> (po, pl)
            pending = None
            epiq = []

            def emit_epilogue():
                c, po, pl = epiq.pop(0)
                qsl = slice(c * CHUNK, (c + 1) * CHUNK)
                ob = osb.tile([D, CHUNK], F32, tag="ob", name="ob")
                nc.vector.tensor_copy(ob, po)
                nc.sync.dma_start(out=ot_d[:, qsl], in_=ob)
                nc.vector.tensor_copy(lvs[:, qsl], pl[0:1, :])
                if c == NCHUNK - 2:
                    # flush chunks 0..6 denominators off the tail early
                    nc.sync.dma_start(out=lv_d[:, 0:(NCHUNK - 1) * CHUNK],
                                      in_=lvs[:, 0:(NCHUNK - 1) * CHUNK])
                elif c == NCHUNK - 1:
                    nc.sync.dma_start(out=lv_d[:, qsl], in_=lvs[:, qsl])

            def emit_av(pend):
                c, pr, los, kind, is_first, is_last, pt = pend
                if c not in acc:
                    acc[c] = (
                        op.tile([D, CHUNK], F32, tag="po", name="po"),
                        lp.tile([D, CHUNK], F32, tag="pl", name="pl"),
                    )
                po, pl = acc[c]
                if kind != "f32":
                    lo = los[0]       # uniform per-pair lo for fp8 kinds
                    v3 = vsb8[:, pr[0] * 128:(pr[0] + 2) * 128].rearrange(
                        "p (k f) -> p k f", k=2)
                    p3 = pt.rearrange("p (k f) -> p k f", k=2)[:, :, lo:]
                    o3 = ones8.rearrange("p (k f) -> p k f", k=2)
                    nc.tensor.matmul(po[:, lo:], v3, p3, start=is_first,
                                     stop=is_last, perf_mode=DR,
                                     skip_group_check=True)
                    nc.tensor.matmul(pl[:, lo:], o3, p3, start=is_first,
                                     stop=is_last, perf_mode=DR,
                                     skip_group_check=True)
                else:
                    for i, t in enumerate(pr):
                        lo = los[i]
                        ptc = pt[:, i * CHUNK + lo:(i + 1) * CHUNK]
                        st_f = is_first and i == 0
                        sp_f = is_last and i == 1
                        nc.tensor.matmul(
                            po[:, lo:], vsb32[:, t * 128:(t + 1) * 128], ptc,
                            start=st_f, stop=sp_f, skip_group_check=True)
                        nc.tensor.matmul(
                            pl[:, lo:], ones32, ptc,
                            start=st_f, stop=sp_f, skip_group_check=True)
                if is_last:
                    epiq.append((c, po, pl))
                    del acc[c]

            for c, pr, los, kind, is_first, is_last in units:
                if is_first and (c + 2) in stage_thunks:
                    thunkq.extend(stage_thunks[c + 2])
                if epiq:
                    emit_epilogue()
                st = sts[nst[0] % 2]
                nst[0] += 1
                for i, t in enumerate(pr):
                    lo = los[i]
                    nc.tensor.matmul(
                        st[:, i * CHUNK + lo:(i + 1) * CHUNK],
                        kt[:, t * 128:(t + 1) * 128],
                        qt[:, c * CHUNK + lo:(c + 1) * CHUNK],
                        start=True, stop=True, skip_group_check=True,
                    )
                    if kind != "f8":
                        # causal mask band over [lo, 128(m+1)): width w
                        # staircase tail, all-NEG before it. msk stores
                        # [allNEG(128) | staircase(128)]; slice the last
                        # w columns.
                        m = t - 4 * c
                        w = 128 * (m + 1) - lo
                        nc.tensor.matmul(
                            st[:, i * CHUNK + lo:i * CHUNK + lo + w],
                            ident, msk[:, 256 - w:256],
                            start=False, stop=True, skip_group_check=True)
                if kind == "f32":
                    pt = pts32[n32[0] % 2]
                    n32[0] += 1
                else:
                    pt = pts8[n8[0] % 3]
                    n8[0] += 1
                # one exp per pair; the inter-tile gap region (columns
                # [CHUNK, CHUNK+los[1]) when los[1] > los[0]) holds stale
                # PSUM, is exp'd harmlessly, and is never read downstream.
                nc.scalar.activation(
                    pt[:, los[0]:], st[:, los[0]:],
                    mybir.ActivationFunctionType.Exp, bias=sh4)
                prev = pending
                pending = (c, pr, los, kind, is_first, is_last, pt)
                if prev is not None:
                    emit_av(prev)
                # projection thunks AFTER the unit's critical ST/exp/AV
                # emissions: the exp stream gets PE priority, thunk matmuls
                # fill the slack
                for _ in range(2):
                    if thunkq:
                        thunkq.pop(0)()
            emit_av(pending)
            while epiq:
                emit_epilogue()

    if legalize:
        _legalize_multiwaits(nc)
    nc.finalize()
    return nc


def _legalize_multiwaits(nc):
    """Hardware instruction structs in this walrus build accept at most ONE
    sync wait. Move all but the last wait onto single-wait same-engine NoOps
    inserted right before the instruction (engines execute in order)."""
    for fn in nc.m.functions:
        for blk in fn.blocks:
            insts = blk.instructions
            out = []
            for inst in insts:
                si = inst.sync_info
                if si is not None and si.on_wait and len(si.on_wait) >= 2:
                    waits = list(si.on_wait)
                    for w in waits[:-1]:
                        out.append(mybir.InstNoOp(
                            name=nc.get_next_instruction_name(),
                            engine=inst.engine,
                            bass_nofuse=True,
                            sync_info=mybir.SyncInfo(
                                on_wait=[w], on_update=[]),
                        ))
                    inst.sync_info = mybir.SyncInfo(
                        on_wait=[waits[-1]],
                        on_update=list(si.on_update or []))
                out.append(inst)
            insts[:] = out


_NC_CACHE = {}


def get_nc(legalize=True):
    key = ("nc", legalize)
    if key not in _NC_CACHE:
        _NC_CACHE[key] = build_nc(legalize)
    return _NC_CACHE[key]


def make_core_inputs(x, Wq, bq, Wk, bk, Wv, bv):
    """Per-core input maps (host-side sharding). bk dropped (softmax
    invariance); bv applied on the host."""
    import ml_dtypes

    s = 1.0 / math.sqrt(D)
    wq16 = (np.asarray(Wq, np.float32) * s).astype(ml_dtypes.bfloat16)
    wk16 = np.asarray(Wk, np.float32).astype(ml_dtypes.bfloat16)
    wv16 = np.asarray(Wv, np.float32).astype(ml_dtypes.bfloat16)

    # msk: cols [0:128) all NEG; [128:256) staircase 0 if j >= k else NEG
    kk = np.arange(128)[:, None]
    jj = np.arange(128)[None, :]
    stair = np.where(jj >= kk, 0.0, NEG).astype(np.float32)
    mskf = np.concatenate([np.full((D, 128), NEG, np.float32), stair], axis=1)
    identf = np.eye(D, dtype=np.float32)
    cb16 = np.concatenate(
        [wq16, wk16, wv16,
         mskf.astype(ml_dtypes.bfloat16), identf.astype(ml_dtypes.bfloat16)],
        axis=1)  # [D, 768] bf16

    cf32 = np.zeros((D, 132), np.float32)
    cf32[:, 0] = np.asarray(bq, np.float32) * s
    cf32[:, 1] = -SHIFT
    cf32[:, 4:132] = 1.0     # f32r all-ones stationary for diag l-matmuls
    ones8 = np.ones((D, 2 * D), ml_dtypes.float8_e4m3)

    x = np.asarray(x, dtype=np.float32)
    in_maps = []
    for core in range(8):
        b, h = core // 2, core % 2
        xb = x[b]
        tri = xb[h * HALF:(h + 1) * HALF]          # [2048, 128]
        rect_q = xb[HALF:]                         # [2048, 128]
        rect_kv = xb[h * 1024:(h + 1) * 1024]      # [1024, 128]
        xin = np.ascontiguousarray(
            np.concatenate([tri, rect_q, rect_kv], axis=0).T
        ).astype(ml_dtypes.bfloat16)               # [128, 5120]
        in_maps.append({"xin": xin, "cb16": cb16, "cf32": cf32,
                        "ones8": ones8})
    return in_maps


def merge_outputs(results, bv):
    """Gather per-core (oT, lv) into the full [B, T, D] output. The -4
    score shift scales o and l identically, so it cancels in o/l."""
    bv = np.asarray(bv, dtype=np.float32)
    out = np.empty((B, T, D), np.float32)
    for b in range(B):
        lo, hi = results[2 * b], results[2 * b + 1]
        lo_lv = np.asarray(lo["lv"]).reshape(NCHUNK, CHUNK)
        hi_lv = np.asarray(hi["lv"]).reshape(NCHUNK, CHUNK)
        O = np.zeros((T, D), np.float64)
        L = np.zeros(T, np.float64)
        O[:HALF] += lo["oT"][:, :HALF].T
        L[:HALF] += lo_lv[0:4].ravel()
        O[HALF:] += hi["oT"][:, :HALF].T
        L[HALF:] += hi_lv[0:4].ravel()
        O[HALF:] += lo["oT"][:, HALF:].T
        L[HALF:] += lo_lv[4:8].ravel()
        O[HALF:] += hi["oT"][:, HALF:].T
        L[HALF:] += hi_lv[4:8].ravel()
        out[b] = (O / L[:, None]).astype(np.float32) + bv
    return out


def run_per_core(nc, in_maps, threads=True):
    """Run the same single-core program on each NeuronCore with its own
    inputs (per-core dispatch; the cores share no collectives)."""
    import jax
    from concourse import bass2jax

    devices = jax.devices()[:len(in_maps)]

    def one(i):
        with jax.default_device(devices[i]):
            return bass2jax.run_bass_via_pjrt(nc, [in_maps[i]], n_cores=1)[0]

    if threads:
        from concurrent.futures import ThreadPoolExecutor
        first = one(0)
        with ThreadPoolExecutor(max_workers=7) as ex:
            rest = list(ex.map(one, range(1, len(in_maps))))
        return [first] + rest
    return [one(i) for i in range(len(in_maps))]


def kernel(x, Wq, bq, Wk, bk, Wv, bv, _trace=False):
    from concourse.bass_utils import axon_active, run_bass_kernel_spmd

    nc = get_nc()
    in_maps = make_core_inputs(x, Wq, bq, Wk, bk, Wv, bv)
    if axon_active():
        results = run_per_core(nc, in_maps)
    else:
        res = run_bass_kernel_spmd(nc, in_maps, list(range(8)), trace=_trace)
        kernel.last_result = res
        results = res.results
    out = merge_outputs(results, bv)
    return out


# revision 45
# speedup vs baseline: 1.4582x; 1.0336x over previous
"""Trainium2 Bass kernel: single-head causal attention (v2).

Problem: x[4,4096,128]; Q/K/V linear projections (W [in,out] layout, +bias);
scores = QK^T/sqrt(128) with causal mask; softmax; out = P @ V.

Sharding (8 cores = 4 batches x 2), SPMD (all cores run one program):
  core (b, h):
    triangle: queries q in [2048h, 2048h+2048) of batch b attending causally
        to kv rows in the same range.
    rectangle: queries q in [2048, 4096) attending to kv [1024h, 1024h+1024)
        (fully valid, no mask).
  Union over both cores of a batch covers the full causal set exactly once.

Softmax without max subtraction but with a constant shift: every score gets
-4.0 before exp (softmax shift-invariance keeps O/l exact), which brings
P = exp(s-4) into fp8 e4m3 range (max ~6 << 240). The cross-core merge stays
linear: host sums unnormalized o and denominators l, then divides.

Numerics by stage (validated to 6.8e-3 rel err vs the fp32 reference):
  - x and W DMA'd in bf16; projections are bf16 matmuls (1 cycle/row always,
    no fp32r narrow-output penalty), PSUM fp32.
  - QT/KT stored f32r; score matmuls (ST) run fp32r (full rate >= 256 wide).
  - Full (unmasked) kv-tile pairs: exp writes P~T in fp8 e4m3; AV and the
    denominator matmul run in DoubleRow perf mode (two 128-kv tiles per
    pass, 0.5 cycles/row = 4x the fp32r rate). V kept in fp8 (vsb8).
  - Diagonal (causally masked) pairs stay f32r end to end: short softmax
    rows have no error averaging, fp8 there fails the gate (5.8e-2).
    Tile m=3 is widened to a 256-col window (fully-masked prefix) so every
    diagonal ST/AV/l matmul is >= 256 wide (no fp32r 4x narrow penalty).
  - bk drops out of softmax; bq (pre-scaled) added during the Q PSUM->SBUF
    copy; bv added by the host after normalization.

Schedule: projections are software-pipelined INTO the attention chunk
stream (one or two projection matmuls between attention units) so the
scalar engine starts exp'ing ~3.5us in and never starves; the exp stream
(~36us on ACT) is the critical resource, with PE at ~33us underneath it.
Mask adds are bf16 identity-matmuls on the PE (no ACT/DVE time). PSUM->
SBUF copies and output staging are split between DVE and GPSIMD(Pool),
keeping ACT exp-only. PSUM banks: st 2x2 + proj 2x1 + po 1 + pl 1 = 8.

Device layouts (per core):
  xin  [128,5120] bf16  x^T cols: 0:4096 = (tri|rect_q), 4096:5120 = rect_kv
  cb16 [128,768]  bf16  wq|wk|wv|msk(256)|ident
  cf32 [128,4]    f32r  bq' | -4.0 | 1.0 (ones) | fp8-ones bit pattern
  QT [128,4096] f32r; KT [128,3072] f32r
  vsb32 [128,2048] f32r (V tiles 0-15, diag pairs)
  vsb8  [128,3072] fp8  (V tiles 0-23, full pairs)
  ST computed transposed: ST[k,q] = K Q^T in PSUM; exp(ST-4) -> P~T in
  fp8 (full pairs) or f32r (diag pairs); AV: oT[e,q] += V^T-mm-P~T;
  l[q] += ones-mm-P~T (PE is the only partition reducer).
Outputs: oT [128,4096] f32 (transposed, unnormalized), lv [1,4096]
(denominators, chunk-major). Host transposes, merges, normalizes, adds bv.
"""

import math
import sys

import numpy as np

sys.path.insert(0, "/opt/trn_rl_repo")

import concourse.bass as bass  # noqa: E402
import concourse.mybir as mybir  # noqa: E402
from concourse.tile import TileContext  # noqa: E402

B, T, D = 4, 4096, 128
HALF = T // 2          # 2048 queries per triangle
NCHUNK = 8             # 8 chunks of 512 query slots (4 tri + 4 rect)
CHUNK = 512
KV_TILES = 24          # 16 tri + 8 rect kv tiles of 128 rows
NEG = -99840.0         # additive mask value, exactly representable in bf16
SHIFT = 4.0            # score shift: exp(s - 4) keeps P in fp8 range

F32 = mybir.dt.float32
F32R = mybir.dt.float32r
BF16 = mybir.dt.bfloat16
F8 = mybir.dt.float8e4
DR = mybir.MatmulPerfMode.DoubleRow


def round_f32r(a):
    """Exact fp32 -> fp32r rounding (RNE to 11 mantissa bits)."""
    u = np.ascontiguousarray(a, np.float32).view(np.uint32)
    add = np.uint32(0x7FF) + ((u >> np.uint32(12)) & np.uint32(1))
    return ((u + add) & np.uint32(0xFFFFF000)).view(np.float32)


def build_nc(legalize=True):
    nc = bass.Bass()

    xin_d = nc.declare_dram_parameter("xin", [D, 5120], BF16, isOutput=False)
    cb16_d = nc.declare_dram_parameter("cb16", [D, 768], BF16, isOutput=False)
    cf32_d = nc.declare_dram_parameter("cf32", [D, 132], F32R, isOutput=False)
    ones8_d = nc.declare_dram_parameter("ones8", [D, 2 * D], F8, isOutput=False)

    ot_d = nc.declare_dram_parameter("oT", [D, T], F32, isOutput=True)
    lv_d = nc.declare_dram_parameter("lv", [1, T], F32, isOutput=True)

    with TileContext(nc) as tc:
        with (
            tc.tile_pool(name="sb", bufs=1) as sb,
            tc.tile_pool(name="stp", bufs=1, space="PSUM") as stp,
            tc.tile_pool(name="pp", bufs=2, space="PSUM") as pp,
            tc.tile_pool(name="op", bufs=1, space="PSUM") as op,
            tc.tile_pool(name="lp", bufs=1, space="PSUM") as lp,
            tc.tile_pool(name="osb", bufs=4) as osb,
        ):
            # ---- input DMAs, critical-path order: stage 0 (weights + xin
            # cols 0:512 + consts) lands first so PE starts ~2.6us in ----
            cb16 = sb.tile([D, 768], BF16)
            nc.sync.dma_start(out=cb16, in_=cb16_d[:, :])
            xin = sb.tile([D, 5120], BF16)
            nc.sync.dma_start(out=xin[:, 0:512], in_=xin_d[:, 0:512])
            cf32 = sb.tile([D, 132], F32R)
            nc.sync.dma_start(out=cf32, in_=cf32_d[:, :])
            nc.sync.dma_start(out=xin[:, 512:1024], in_=xin_d[:, 512:1024])
            nc.sync.dma_start(out=xin[:, 1024:2048], in_=xin_d[:, 1024:2048])
            ones8 = sb.tile([D, 2 * D], F8)
            nc.sync.dma_start(out=ones8, in_=ones8_d[:, :])
            nc.sync.dma_start(out=xin[:, 4096:5120], in_=xin_d[:, 4096:5120])
            nc.sync.dma_start(out=xin[:, 2048:3072], in_=xin_d[:, 2048:3072])
            nc.sync.dma_start(out=xin[:, 3072:4096], in_=xin_d[:, 3072:4096])

            wq = cb16[:, 0:128]
            wk = cb16[:, 128:256]
            wv = cb16[:, 256:384]
            msk = cb16[:, 384:640]     # [0:128) all NEG | [128:256) staircase
            ident = cb16[:, 640:768]
            bq = cf32[:, 0:1].bitcast(F32)
            sh4 = cf32[:, 1:2].bitcast(F32)  # -4.0 exp bias
            # l-matmul stationaries: 128-wide all-ones (the ISA rejects
            # narrow DoubleRow ldweights; a full-width stationary costs the
            # same moving columns and makes every PSUM row a copy of l)
            ones32 = cf32[:, 4:132]

            qt = sb.tile([D, T], F32R)
            kt = sb.tile([D, KV_TILES * 128], F32R)
            vsb32 = sb.tile([D, CHUNK], F32R)   # V tiles 0-3 (chunk-0 diag)
            vsb8 = sb.tile([D, KV_TILES * 128], F8)
            lvs = sb.tile([1, T], F32)

            # ---- projection stages (pipelined into the attention stream).
            # Copies alternate DVE / GPSIMD so ACT stays exp-only. ----
            def xcol(t):  # xin column of kv tile t
                return t * 128 if t < 16 else 4096 + (t - 16) * 128

            def v_stage(g, pool=None):   # V tiles 4g..4g+3 -> [kvrow, e]
                ps = (pool or pp).tile(
                    [D, CHUNK], F32,
                    **({"tag": "po", "name": "po"} if pool is not None
                       else {"tag": "pp", "name": f"psv{g}"}))
                for jj in range(4):
                    t = 4 * g + jj
                    nc.tensor.matmul(
                        ps[:, jj * 128:(jj + 1) * 128],
                        xin[:, xcol(t):xcol(t) + 128], wv,
                        start=True, stop=True, skip_group_check=True,
                    )
                # GPSIMD cannot touch PSUM: DVE drains the bank, GPSIMD does
                # the off-critical-path SBUF->SBUF fp8 conversion.
                sl = slice(g * CHUNK, (g + 1) * CHUNK)
                if g == 0:
                    nc.vector.tensor_copy(vsb32, ps)
                    nc.gpsimd.tensor_copy(vsb8[:, sl], vsb32)
                else:
                    nc.vector.tensor_copy(vsb8[:, sl], ps)

            def k_stage(j, pool=None, act=False):   # K^T chunk j
                ps = (pool or pp).tile(
                    [D, CHUNK], F32,
                    **({"tag": "pl", "name": "pl"} if pool is not None
                       else {"tag": "pp", "name": f"psk{j}"}))
                src = xin[:, j * CHUNK:(j + 1) * CHUNK] if j < 4 else \
                    xin[:, 4096 + (j - 4) * CHUNK:4096 + (j - 3) * CHUNK]
                nc.tensor.matmul(ps, wk, src,
                                 start=True, stop=True, skip_group_check=True)
                if act:
                    nc.scalar.copy(kt[:, j * CHUNK:(j + 1) * CHUNK], ps)
                else:
                    nc.vector.tensor_copy(
                        kt[:, j * CHUNK:(j + 1) * CHUNK], ps)

            def q_stage(c):          # Q^T chunk c (scaled, biased)
                ps = pp.tile([D, CHUNK], F32, tag="pp", name=f"psq{c}")
                nc.tensor.matmul(ps, wq, xin[:, c * CHUNK:(c + 1) * CHUNK],
                                 start=True, stop=True, skip_group_check=True)
                if c == 0:
                    # ACT is idle before the first exp; the chunk-0
                    # bias-copy there unblocks DVE for the k/v copies
                    nc.scalar.activation(
                        qt[:, c * CHUNK:(c + 1) * CHUNK], ps,
                        mybir.ActivationFunctionType.Identity, bias=bq)
                else:
                    nc.vector.tensor_scalar_add(
                        qt[:, c * CHUNK:(c + 1) * CHUNK], ps, bq)

            # two persistent score buffers, manually rotated. The merged
            # diag exp reads a small inter-tile gap; only chunk 0's first
            # use of each buffer sees it uninitialized (stale-but-finite
            # afterwards, never consumed) - zero exactly those windows.
            sts = [stp.tile([D, 2 * CHUNK], F32, name=f"st{i}")
                   for i in range(2)]
            nc.vector.memset(sts[0][:, CHUNK:CHUNK + 128], 0.0)
            nc.vector.memset(sts[1][:, CHUNK:CHUNK + 256], 0.0)
            nst = [0]

            # prologue: only what chunk 0 needs, on FOUR parallel PSUM
            # banks (op/lp are idle until the first AV, so the v0/k1
            # projections borrow them -> no copy->matmul WAR chain).
            # k0's copy rides the still-idle ACT engine.
            q_stage(0)
            k_stage(0, act=True)
            v_stage(0, pool=op)
            k_stage(1, pool=lp)
            thunkq = [lambda: q_stage(1), lambda: v_stage(1)]
            stage_thunks = {
                2: [lambda: v_stage(2), lambda: k_stage(2), lambda: q_stage(2)],
                3: [lambda: v_stage(3), lambda: k_stage(3), lambda: q_stage(3)],
                4: [lambda: v_stage(4), lambda: v_stage(5), lambda: k_stage(4),
                    lambda: k_stage(5), lambda: q_stage(4)],
                5: [lambda: q_stage(5)],
                6: [lambda: q_stage(6)],
                7: [lambda: q_stage(7)],
            }

            # ---- attention: 8 chunks; units are kv-tile pairs.
            # Tri chunks: 2 diagonal (masked) pairs first, then full pairs
            # (fp8 DoubleRow) descending. Only chunk 0's diagonals (the
            # short softmax rows, no error averaging) stay f32r; chunks
            # 1-3 diagonals run fp8 DoubleRow with a uniform per-pair lo.
            # Skew-1 software pipeline: AV+l of unit u are emitted after
            # ST/exp of unit u+1. Kinds: 'f32' chunk-0 diag, 'f8d' fp8
            # diag (masked), 'f8' full. ----
            units = []
            for c in range(NCHUNK):
                if c == 0:
                    pairs = [((0, 1), (0, 128), "f32"),
                             ((2, 3), (256, 256), "f32")]
                elif c < 4:
                    pairs = [((4 * c, 4 * c + 1), (0, 0), "f8d"),
                             ((4 * c + 2, 4 * c + 3), (256, 256), "f8d")]
                    for t0 in range(4 * c - 2, -1, -2):
                        pairs.append(((t0, t0 + 1), (0, 0), "f8"))
                else:
                    pairs = [((16 + 2 * i, 17 + 2 * i), (0, 0), "f8")
                             for i in range(4)]
                for pi, (pr, los, kind) in enumerate(pairs):
                    units.append((c, pr, los, kind, pi == 0,
                                  pi == len(pairs) - 1))

            pts8 = [sb.tile([D, 2 * CHUNK], F8, name=f"pt8_{i}")
                    for i in range(3)]
            pts32 = [sb.tile([D, 2 * CHUNK], F32R, name=f"pt32_{i}")
                     for i in range(2)]
            n8 = [0]
            n32 = [0]
            acc = {}                # chunk -> (po, pl)
            pending = None
            epiq = []

            def emit_epilogue():
                c, po, pl = epiq.pop(0)
                qsl = slice(c * CHUNK, (c + 1) * CHUNK)
                ob = osb.tile([D, CHUNK], F32, tag="ob", name="ob")
                nc.vector.tensor_copy(ob, po)
                nc.sync.dma_start(out=ot_d[:, qsl], in_=ob)
                nc.vector.tensor_copy(lvs[:, qsl], pl[0:1, :])
                if c == NCHUNK - 2:
                    # flush chunks 0..6 denominators off the tail early
                    nc.sync.dma_start(out=lv_d[:, 0:(NCHUNK - 1) * CHUNK],
                                      in_=lvs[:, 0:(NCHUNK - 1) * CHUNK])
                elif c == NCHUNK - 1:
                    nc.sync.dma_start(out=lv_d[:, qsl], in_=lvs[:, qsl])

            def emit_av(pend):
                c, pr, los, kind, is_first, is_last, pt = pend
                if c not in acc:
                    acc[c] = (
                        op.tile([D, CHUNK], F32, tag="po", name="po"),
                        lp.tile([D, CHUNK], F32, tag="pl", name="pl"),
                    )
                po, pl = acc[c]
                if kind != "f32":
                    lo = los[0]       # uniform per-pair lo for fp8 kinds
                    v3 = vsb8[:, pr[0] * 128:(pr[0] + 2) * 128].rearrange(
                        "p (k f) -> p k f", k=2)
                    p3 = pt.rearrange("p (k f) -> p k f", k=2)[:, :, lo:]
                    o3 = ones8.rearrange("p (k f) -> p k f", k=2)
                    nc.tensor.matmul(po[:, lo:], v3, p3, start=is_first,
                                     stop=is_last, perf_mode=DR,
                                     skip_group_check=True)
                    nc.tensor.matmul(pl[:, lo:], o3, p3, start=is_first,
                                     stop=is_last, perf_mode=DR,
                                     skip_group_check=True)
                else:
                    for i, t in enumerate(pr):
                        lo = los[i]
                        ptc = pt[:, i * CHUNK + lo:(i + 1) * CHUNK]
                        st_f = is_first and i == 0
                        sp_f = is_last and i == 1
                        nc.tensor.matmul(
                            po[:, lo:], vsb32[:, t * 128:(t + 1) * 128], ptc,
                            start=st_f, stop=sp_f, skip_group_check=True)
                        nc.tensor.matmul(
                            pl[:, lo:], ones32, ptc,
                            start=st_f, stop=sp_f, skip_group_check=True)
                if is_last:
                    epiq.append((c, po, pl))
                    del acc[c]

            for c, pr, los, kind, is_first, is_last in units:
                if is_first and (c + 2) in stage_thunks:
                    thunkq.extend(stage_thunks[c + 2])
                if epiq:
                    emit_epilogue()
                st = sts[nst[0] % 2]
                nst[0] += 1
                for i, t in enumerate(pr):
                    lo = los[i]
                    nc.tensor.matmul(
                        st[:, i * CHUNK + lo:(i + 1) * CHUNK],
                        kt[:, t * 128:(t + 1) * 128],
                        qt[:, c * CHUNK + lo:(c + 1) * CHUNK],
                        start=True, stop=True, skip_group_check=True,
                    )
                    if kind != "f8":
                        # causal mask band over [lo, 128(m+1)): width w
                        # staircase tail, all-NEG before it. msk stores
                        # [allNEG(128) | staircase(128)]; slice the last
                        # w columns.
                        m = t - 4 * c
                        w = 128 * (m + 1) - lo
                        nc.tensor.matmul(
                            st[:, i * CHUNK + lo:i * CHUNK + lo + w],
                            ident, msk[:, 256 - w:256],
                            start=False, stop=True, skip_group_check=True)
                if kind == "f32":
                    pt = pts32[n32[0] % 2]
                    n32[0] += 1
                else:
                    pt = pts8[n8[0] % 3]
                    n8[0] += 1
                # one exp per pair; the inter-tile gap region (columns
                # [CHUNK, CHUNK+los[1]) when los[1] > los[0]) holds stale
                # PSUM, is exp'd harmlessly, and is never read downstream.
                nc.scalar.activation(
                    pt[:, los[0]:], st[:, los[0]:],
                    mybir.ActivationFunctionType.Exp, bias=sh4)
                prev = pending
                pending = (c, pr, los, kind, is_first, is_last, pt)
                if prev is not None:
                    emit_av(prev)
                # projection thunks AFTER the unit's critical ST/exp/AV
                # emissions: the exp stream gets PE priority, thunk matmuls
                # fill the slack
                for _ in range(2):
                    if thunkq:
                        thunkq.pop(0)()
            emit_av(pending)
            while epiq:
                emit_epilogue()

    if legalize:
        _legalize_multiwaits(nc)
    nc.finalize()
    return nc


def _legalize_multiwaits(nc):
    """Hardware instruction structs in this walrus build accept at most ONE
    sync wait. Move all but the last wait onto single-wait same-engine NoOps
    inserted right before the instruction (engines execute in order)."""
    for fn in nc.m.functions:
        for blk in fn.blocks:
            insts = blk.instructions
            out = []
            for inst in insts:
                si = inst.sync_info
                if si is not None and si.on_wait and len(si.on_wait) >= 2:
                    waits = list(si.on_wait)
                    for w in waits[:-1]:
                        out.append(mybir.InstNoOp(
                            name=nc.get_next_instruction_name(),
                            engine=inst.engine,
                            bass_nofuse=True,
                            sync_info=mybir.SyncInfo(
                                on_wait=[w], on_update=[]),
                        ))
                    inst.sync_info = mybir.SyncInfo(
                        on_wait=[waits[-1]],
                        on_update=list(si.on_update or []))
                out.append(inst)
            insts[:] = out


_NC_CACHE = {}


def get_nc(legalize=True):
    key = ("nc", legalize)
    if key not in _NC_CACHE:
        _NC_CACHE[key] = build_nc(legalize)
    return _NC_CACHE[key]


def make_core_inputs(x, Wq, bq, Wk, bk, Wv, bv):
    """Per-core input maps (host-side sharding). bk dropped (softmax
    invariance); bv applied on the host."""
    import ml_dtypes

    s = 1.0 / math.sqrt(D)
    wq16 = (np.asarray(Wq, np.float32) * s).astype(ml_dtypes.bfloat16)
    wk16 = np.asarray(Wk, np.float32).astype(ml_dtypes.bfloat16)
    wv16 = np.asarray(Wv, np.float32).astype(ml_dtypes.bfloat16)

    # msk: cols [0:128) all NEG; [128:256) staircase 0 if j >= k else NEG
    kk = np.arange(128)[:, None]
    jj = np.arange(128)[None, :]
    stair = np.where(jj >= kk, 0.0, NEG).astype(np.float32)
    mskf = np.concatenate([np.full((D, 128), NEG, np.float32), stair], axis=1)
    identf = np.eye(D, dtype=np.float32)
    cb16 = np.concatenate(
        [wq16, wk16, wv16,
         mskf.astype(ml_dtypes.bfloat16), identf.astype(ml_dtypes.bfloat16)],
        axis=1)  # [D, 768] bf16

    cf32 = np.zeros((D, 132), np.float32)
    cf32[:, 0] = np.asarray(bq, np.float32) * s
    cf32[:, 1] = -SHIFT
    cf32[:, 4:132] = 1.0     # f32r all-ones stationary for diag l-matmuls
    ones8 = np.ones((D, 2 * D), ml_dtypes.float8_e4m3)

    x = np.asarray(x, dtype=np.float32)
    in_maps = []
    for core in range(8):
        b, h = core // 2, core % 2
        xb = x[b]
        tri = xb[h * HALF:(h + 1) * HALF]          # [2048, 128]
        rect_q = xb[HALF:]                         # [2048, 128]
        rect_kv = xb[h * 1024:(h + 1) * 1024]      # [1024, 128]
        xin = np.ascontiguousarray(
            np.concatenate([tri, rect_q, rect_kv], axis=0).T
        ).astype(ml_dtypes.bfloat16)               # [128, 5120]
        in_maps.append({"xin": xin, "cb16": cb16, "cf32": cf32,
                        "ones8": ones8})
    return in_maps


def merge_outputs(results, bv):
    """Gather per-core (oT, lv) into the full [B, T, D] output. The -4
    score shift scales o and l identically, so it cancels in o/l."""
    bv = np.asarray(bv, dtype=np.float32)
    out = np.empty((B, T, D), np.float32)
    for b in range(B):
        lo, hi = results[2 * b], results[2 * b + 1]
        lo_lv = np.asarray(lo["lv"]).reshape(NCHUNK, CHUNK)
        hi_lv = np.asarray(hi["lv"]).reshape(NCHUNK, CHUNK)
        O = np.zeros((T, D), np.float64)
        L = np.zeros(T, np.float64)
        O[:HALF] += lo["oT"][:, :HALF].T
        L[:HALF] += lo_lv[0:4].ravel()
        O[HALF:] += hi["oT"][:, :HALF].T
        L[HALF:] += hi_lv[0:4].ravel()
        O[HALF:] += lo["oT"][:, HALF:].T
        L[HALF:] += lo_lv[4:8].ravel()
        O[HALF:] += hi["oT"][:, HALF:].T
        L[HALF:] += hi_lv[4:8].ravel()
        out[b] = (O / L[:, None]).astype(np.float32) + bv
    return out


def run_per_core(nc, in_maps, threads=True):
    """Run the same single-core program on each NeuronCore with its own
    inputs (per-core dispatch; the cores share no collectives)."""
    import jax
    from concourse import bass2jax

    devices = jax.devices()[:len(in_maps)]

    def one(i):
        with jax.default_device(devices[i]):
            return bass2jax.run_bass_via_pjrt(nc, [in_maps[i]], n_cores=1)[0]

    if threads:
        from concurrent.futures import ThreadPoolExecutor
        first = one(0)
        with ThreadPoolExecutor(max_workers=7) as ex:
            rest = list(ex.map(one, range(1, len(in_maps))))
        return [first] + rest
    return [one(i) for i in range(len(in_maps))]


def kernel(x, Wq, bq, Wk, bk, Wv, bv, _trace=False):
    from concourse.bass_utils import axon_active, run_bass_kernel_spmd

    nc = get_nc()
    in_maps = make_core_inputs(x, Wq, bq, Wk, bk, Wv, bv)
    if axon_active():
        results = run_per_core(nc, in_maps)
    else:
        res = run_bass_kernel_spmd(nc, in_maps, list(range(8)), trace=_trace)
        kernel.last_result = res
        results = res.results
    out = merge_outputs(results, bv)
    return out


# revision 46
# speedup vs baseline: 1.4755x; 1.0118x over previous
"""Trainium2 Bass kernel: single-head causal attention (v2).

Problem: x[4,4096,128]; Q/K/V linear projections (W [in,out] layout, +bias);
scores = QK^T/sqrt(128) with causal mask; softmax; out = P @ V.

Sharding (8 cores = 4 batches x 2), SPMD (all cores run one program):
  core (b, h):
    triangle: queries q in [2048h, 2048h+2048) of batch b attending causally
        to kv rows in the same range.
    rectangle: queries q in [2048, 4096) attending to kv [1024h, 1024h+1024)
        (fully valid, no mask).
  Union over both cores of a batch covers the full causal set exactly once.

Softmax without max subtraction but with a constant shift: every score gets
-4.0 before exp (softmax shift-invariance keeps O/l exact), which brings
P = exp(s-4) into fp8 e4m3 range (max ~6 << 240). The cross-core merge stays
linear: host sums unnormalized o and denominators l, then divides.

Numerics by stage (validated to 6.8e-3 rel err vs the fp32 reference):
  - x and W DMA'd in bf16; projections are bf16 matmuls (1 cycle/row always,
    no fp32r narrow-output penalty), PSUM fp32.
  - QT/KT stored f32r; score matmuls (ST) run fp32r (full rate >= 256 wide).
  - Full (unmasked) kv-tile pairs: exp writes P~T in fp8 e4m3; AV and the
    denominator matmul run in DoubleRow perf mode (two 128-kv tiles per
    pass, 0.5 cycles/row = 4x the fp32r rate). V kept in fp8 (vsb8).
  - Diagonal (causally masked) pairs stay f32r end to end: short softmax
    rows have no error averaging, fp8 there fails the gate (5.8e-2).
    Tile m=3 is widened to a 256-col window (fully-masked prefix) so every
    diagonal ST/AV/l matmul is >= 256 wide (no fp32r 4x narrow penalty).
  - bk drops out of softmax; bq (pre-scaled) added during the Q PSUM->SBUF
    copy; bv added by the host after normalization.

Schedule: projections are software-pipelined INTO the attention chunk
stream (one or two projection matmuls between attention units) so the
scalar engine starts exp'ing ~3.5us in and never starves; the exp stream
(~36us on ACT) is the critical resource, with PE at ~33us underneath it.
Mask adds are bf16 identity-matmuls on the PE (no ACT/DVE time). PSUM->
SBUF copies and output staging are split between DVE and GPSIMD(Pool),
keeping ACT exp-only. PSUM banks: st 2x2 + proj 2x1 + po 1 + pl 1 = 8.

Device layouts (per core):
  xin  [128,5120] bf16  x^T cols: 0:4096 = (tri|rect_q), 4096:5120 = rect_kv
  cb16 [128,768]  bf16  wq|wk|wv|msk(256)|ident
  cf32 [128,4]    f32r  bq' | -4.0 | 1.0 (ones) | fp8-ones bit pattern
  QT [128,4096] f32r; KT [128,3072] f32r
  vsb32 [128,2048] f32r (V tiles 0-15, diag pairs)
  vsb8  [128,3072] fp8  (V tiles 0-23, full pairs)
  ST computed transposed: ST[k,q] = K Q^T in PSUM; exp(ST-4) -> P~T in
  fp8 (full pairs) or f32r (diag pairs); AV: oT[e,q] += V^T-mm-P~T;
  l[q] += ones-mm-P~T (PE is the only partition reducer).
Outputs: oT [128,4096] f32 (transposed, unnormalized), lv [1,4096]
(denominators, chunk-major). Host transposes, merges, normalizes, adds bv.
"""

import math
import sys

import numpy as np

sys.path.insert(0, "/opt/trn_rl_repo")

import concourse.bass as bass  # noqa: E402
import concourse.mybir as mybir  # noqa: E402
from concourse.tile import TileContext  # noqa: E402

B, T, D = 4, 4096, 128
HALF = T // 2          # 2048 queries per triangle
NCHUNK = 8             # 8 chunks of 512 query slots (4 tri + 4 rect)
CHUNK = 512
KV_TILES = 24          # 16 tri + 8 rect kv tiles of 128 rows
NEG = -99840.0         # additive mask value, exactly representable in bf16
SHIFT = 4.0            # score shift: exp(s - 4) keeps P in fp8 range

F32 = mybir.dt.float32
F32R = mybir.dt.float32r
BF16 = mybir.dt.bfloat16
F8 = mybir.dt.float8e4
DR = mybir.MatmulPerfMode.DoubleRow


def round_f32r(a):
    """Exact fp32 -> fp32r rounding (RNE to 11 mantissa bits)."""
    u = np.ascontiguousarray(a, np.float32).view(np.uint32)
    add = np.uint32(0x7FF) + ((u >> np.uint32(12)) & np.uint32(1))
    return ((u + add) & np.uint32(0xFFFFF000)).view(np.float32)


def build_nc(legalize=True):
    nc = bass.Bass()

    xin_d = nc.declare_dram_parameter("xin", [D, 5120], BF16, isOutput=False)
    cb16_d = nc.declare_dram_parameter("cb16", [D, 768], BF16, isOutput=False)
    cf32_d = nc.declare_dram_parameter("cf32", [D, 132], F32R, isOutput=False)
    ones8_d = nc.declare_dram_parameter("ones8", [D, 2 * D], F8, isOutput=False)

    ot_d = nc.declare_dram_parameter("oT", [D, T], F32, isOutput=True)
    lv_d = nc.declare_dram_parameter("lv", [1, T], F32, isOutput=True)

    with TileContext(nc) as tc:
        with (
            tc.tile_pool(name="sb", bufs=1) as sb,
            tc.tile_pool(name="stp", bufs=1, space="PSUM") as stp,
            tc.tile_pool(name="pp", bufs=2, space="PSUM") as pp,
            tc.tile_pool(name="op", bufs=1, space="PSUM") as op,
            tc.tile_pool(name="lp", bufs=1, space="PSUM") as lp,
            tc.tile_pool(name="osb", bufs=4) as osb,
        ):
            # ---- input DMAs, critical-path order: stage 0 (weights + xin
            # cols 0:512 + consts) lands first so PE starts ~2.6us in ----
            cb16 = sb.tile([D, 768], BF16)
            nc.sync.dma_start(out=cb16, in_=cb16_d[:, :])
            xin = sb.tile([D, 5120], BF16)
            nc.sync.dma_start(out=xin[:, 0:512], in_=xin_d[:, 0:512])
            cf32 = sb.tile([D, 132], F32R)
            nc.sync.dma_start(out=cf32, in_=cf32_d[:, :])
            nc.sync.dma_start(out=xin[:, 512:1024], in_=xin_d[:, 512:1024])
            nc.sync.dma_start(out=xin[:, 1024:2048], in_=xin_d[:, 1024:2048])
            ones8 = sb.tile([D, 2 * D], F8)
            nc.sync.dma_start(out=ones8, in_=ones8_d[:, :])
            nc.sync.dma_start(out=xin[:, 4096:5120], in_=xin_d[:, 4096:5120])
            nc.sync.dma_start(out=xin[:, 2048:3072], in_=xin_d[:, 2048:3072])
            nc.sync.dma_start(out=xin[:, 3072:4096], in_=xin_d[:, 3072:4096])

            wq = cb16[:, 0:128]
            wk = cb16[:, 128:256]
            wv = cb16[:, 256:384]
            msk = cb16[:, 384:640]     # [0:128) all NEG | [128:256) staircase
            ident = cb16[:, 640:768]
            bq = cf32[:, 0:1].bitcast(F32)
            sh4 = cf32[:, 1:2].bitcast(F32)  # -4.0 exp bias
            # l-matmul stationaries: 128-wide all-ones (the ISA rejects
            # narrow DoubleRow ldweights; a full-width stationary costs the
            # same moving columns and makes every PSUM row a copy of l)
            ones32 = cf32[:, 4:132]

            qt = sb.tile([D, T], F32R)
            kt = sb.tile([D, KV_TILES * 128], F32R)
            vsb32 = sb.tile([D, CHUNK], F32R)   # V tiles 0-3 (chunk-0 diag)
            vsb8 = sb.tile([D, KV_TILES * 128], F8)
            lvs = sb.tile([1, T], F32)

            # ---- projection stages (pipelined into the attention stream).
            # Copies alternate DVE / GPSIMD so ACT stays exp-only. ----
            def xcol(t):  # xin column of kv tile t
                return t * 128 if t < 16 else 4096 + (t - 16) * 128

            def v_stage(g, pool=None):   # V tiles 4g..4g+3 -> [kvrow, e]
                ps = (pool or pp).tile(
                    [D, CHUNK], F32,
                    **({"tag": "po", "name": "po"} if pool is not None
                       else {"tag": "pp", "name": f"psv{g}"}))
                for jj in range(4):
                    t = 4 * g + jj
                    nc.tensor.matmul(
                        ps[:, jj * 128:(jj + 1) * 128],
                        xin[:, xcol(t):xcol(t) + 128], wv,
                        start=True, stop=True, skip_group_check=True,
                    )
                # GPSIMD cannot touch PSUM: DVE drains the bank, GPSIMD does
                # the off-critical-path SBUF->SBUF fp8 conversion.
                sl = slice(g * CHUNK, (g + 1) * CHUNK)
                if g == 0:
                    nc.vector.tensor_copy(vsb32, ps)
                    nc.gpsimd.tensor_copy(vsb8[:, sl], vsb32)
                else:
                    nc.vector.tensor_copy(vsb8[:, sl], ps)

            def k_stage(j, pool=None, act=False):   # K^T chunk j
                ps = (pool or pp).tile(
                    [D, CHUNK], F32,
                    **({"tag": "pl", "name": "pl"} if pool is not None
                       else {"tag": "pp", "name": f"psk{j}"}))
                src = xin[:, j * CHUNK:(j + 1) * CHUNK] if j < 4 else \
                    xin[:, 4096 + (j - 4) * CHUNK:4096 + (j - 3) * CHUNK]
                nc.tensor.matmul(ps, wk, src,
                                 start=True, stop=True, skip_group_check=True)
                if act:
                    nc.scalar.copy(kt[:, j * CHUNK:(j + 1) * CHUNK], ps)
                else:
                    nc.vector.tensor_copy(
                        kt[:, j * CHUNK:(j + 1) * CHUNK], ps)

            def q_stage(c):          # Q^T chunk c (scaled, biased)
                ps = pp.tile([D, CHUNK], F32, tag="pp", name=f"psq{c}")
                nc.tensor.matmul(ps, wq, xin[:, c * CHUNK:(c + 1) * CHUNK],
                                 start=True, stop=True, skip_group_check=True)
                if c == 0:
                    # ACT is idle before the first exp; the chunk-0
                    # bias-copy there unblocks DVE for the k/v copies
                    nc.scalar.activation(
                        qt[:, c * CHUNK:(c + 1) * CHUNK], ps,
                        mybir.ActivationFunctionType.Identity, bias=bq)
                else:
                    nc.vector.tensor_scalar_add(
                        qt[:, c * CHUNK:(c + 1) * CHUNK], ps, bq)

            # two persistent score buffers, manually rotated. The merged
            # diag exp reads a small inter-tile gap; only chunk 0's first
            # use of each buffer sees it uninitialized (stale-but-finite
            # afterwards, never consumed) - zero exactly those windows.
            sts = [stp.tile([D, 2 * CHUNK], F32, name=f"st{i}")
                   for i in range(2)]
            nc.vector.memset(sts[0][:, CHUNK:CHUNK + 128], 0.0)
            nc.vector.memset(sts[1][:, CHUNK:CHUNK + 256], 0.0)
            nst = [0]

            # prologue: only what chunk 0 needs, on FOUR parallel PSUM
            # banks (op/lp are idle until the first AV, so the v0/k1
            # projections borrow them -> no copy->matmul WAR chain).
            # k0's copy rides the still-idle ACT engine.
            q_stage(0)
            k_stage(0, act=True)
            v_stage(0, pool=op)
            k_stage(1, pool=lp)
            thunkq = [lambda: q_stage(1), lambda: v_stage(1)]
            stage_thunks = {
                2: [lambda: v_stage(2), lambda: k_stage(2), lambda: q_stage(2)],
                3: [lambda: v_stage(3), lambda: k_stage(3), lambda: q_stage(3)],
                4: [lambda: v_stage(4), lambda: v_stage(5), lambda: k_stage(4),
                    lambda: k_stage(5), lambda: q_stage(4)],
                5: [lambda: q_stage(5)],
                6: [lambda: q_stage(6)],
                7: [lambda: q_stage(7)],
            }

            # ---- attention: 8 chunks; units are kv-tile pairs.
            # Tri chunks: 2 diagonal (masked) pairs first, then full pairs
            # (fp8 DoubleRow) descending. Only chunk 0's diagonals (the
            # short softmax rows, no error averaging) stay f32r; chunks
            # 1-3 diagonals run fp8 DoubleRow with a uniform per-pair lo.
            # Skew-1 software pipeline: AV+l of unit u are emitted after
            # ST/exp of unit u+1. Kinds: 'f32' chunk-0 diag, 'f8d' fp8
            # diag (masked), 'f8' full. ----
            units = []
            for c in range(NCHUNK):
                if c == 0:
                    pairs = [((0, 1), (0, 128), "f32"),
                             ((2, 3), (256, 256), "f32")]
                elif c < 4:
                    pairs = [((4 * c, 4 * c + 1), (0, 0), "f8d"),
                             ((4 * c + 2, 4 * c + 3), (256, 256), "f8d")]
                    for t0 in range(4 * c - 2, -1, -2):
                        pairs.append(((t0, t0 + 1), (0, 0), "f8"))
                else:
                    pairs = [((16 + 2 * i, 17 + 2 * i), (0, 0), "f8")
                             for i in range(4)]
                for pi, (pr, los, kind) in enumerate(pairs):
                    units.append((c, pr, los, kind, pi == 0,
                                  pi == len(pairs) - 1))

            pts8 = [sb.tile([D, 2 * CHUNK], F8, name=f"pt8_{i}")
                    for i in range(3)]
            pts32 = [sb.tile([D, 2 * CHUNK], F32R, name=f"pt32_{i}")
                     for i in range(2)]
            n8 = [0]
            n32 = [0]
            acc = {}                # chunk -> (po, pl)
            pending = None
            epiq = []

            def emit_epilogue():
                c, po, pl = epiq.pop(0)
                qsl = slice(c * CHUNK, (c + 1) * CHUNK)
                ob = osb.tile([D, CHUNK], F32, tag="ob", name="ob")
                nc.vector.tensor_copy(ob, po)
                nc.sync.dma_start(out=ot_d[:, qsl], in_=ob)
                nc.vector.tensor_copy(lvs[:, qsl], pl[0:1, :])
                if c == NCHUNK - 2:
                    # flush chunks 0..6 denominators off the tail early
                    nc.sync.dma_start(out=lv_d[:, 0:(NCHUNK - 1) * CHUNK],
                                      in_=lvs[:, 0:(NCHUNK - 1) * CHUNK])
                elif c == NCHUNK - 1:
                    nc.sync.dma_start(out=lv_d[:, qsl], in_=lvs[:, qsl])

            def emit_av(pend):
                c, pr, los, kind, is_first, is_last, pt = pend
                if c not in acc:
                    acc[c] = (
                        op.tile([D, CHUNK], F32, tag="po", name="po"),
                        lp.tile([D, CHUNK], F32, tag="pl", name="pl"),
                    )
                po, pl = acc[c]
                if kind != "f32":
                    lo = los[0]       # uniform per-pair lo for fp8 kinds
                    v3 = vsb8[:, pr[0] * 128:(pr[0] + 2) * 128].rearrange(
                        "p (k f) -> p k f", k=2)
                    p3 = pt.rearrange("p (k f) -> p k f", k=2)[:, :, lo:]
                    o3 = ones8.rearrange("p (k f) -> p k f", k=2)
                    nc.tensor.matmul(po[:, lo:], v3, p3, start=is_first,
                                     stop=is_last, perf_mode=DR,
                                     skip_group_check=True)
                    nc.tensor.matmul(pl[:, lo:], o3, p3, start=is_first,
                                     stop=is_last, perf_mode=DR,
                                     skip_group_check=True)
                else:
                    for i, t in enumerate(pr):
                        lo = los[i]
                        ptc = pt[:, i * CHUNK + lo:(i + 1) * CHUNK]
                        st_f = is_first and i == 0
                        sp_f = is_last and i == 1
                        nc.tensor.matmul(
                            po[:, lo:], vsb32[:, t * 128:(t + 1) * 128], ptc,
                            start=st_f, stop=sp_f, skip_group_check=True)
                        nc.tensor.matmul(
                            pl[:, lo:], ones32, ptc,
                            start=st_f, stop=sp_f, skip_group_check=True)
                if is_last:
                    epiq.append((c, po, pl))
                    del acc[c]

            for c, pr, los, kind, is_first, is_last in units:
                if is_first and (c + 2) in stage_thunks:
                    thunkq.extend(stage_thunks[c + 2])
                if epiq:
                    emit_epilogue()
                st = sts[nst[0] % 2]
                nst[0] += 1
                for i, t in enumerate(pr):
                    lo = los[i]
                    nc.tensor.matmul(
                        st[:, i * CHUNK + lo:(i + 1) * CHUNK],
                        kt[:, t * 128:(t + 1) * 128],
                        qt[:, c * CHUNK + lo:(c + 1) * CHUNK],
                        start=True, stop=True, skip_group_check=True,
                    )
                    if kind != "f8":
                        # causal mask band over [lo, 128(m+1)): width w
                        # staircase tail, all-NEG before it. msk stores
                        # [allNEG(128) | staircase(128)]; slice the last
                        # w columns.
                        m = t - 4 * c
                        w = 128 * (m + 1) - lo
                        nc.tensor.matmul(
                            st[:, i * CHUNK + lo:i * CHUNK + lo + w],
                            ident, msk[:, 256 - w:256],
                            start=False, stop=True, skip_group_check=True)
                if kind == "f32":
                    pt = pts32[n32[0] % 2]
                    n32[0] += 1
                else:
                    pt = pts8[n8[0] % 3]
                    n8[0] += 1
                # one exp per pair; the inter-tile gap region (columns
                # [CHUNK, CHUNK+los[1]) when los[1] > los[0]) holds stale
                # PSUM, is exp'd harmlessly, and is never read downstream.
                nc.scalar.activation(
                    pt[:, los[0]:], st[:, los[0]:],
                    mybir.ActivationFunctionType.Exp, bias=sh4)
                prev = pending
                pending = (c, pr, los, kind, is_first, is_last, pt)
                if prev is not None:
                    emit_av(prev)
                # projection thunks AFTER the unit's critical ST/exp/AV
                # emissions, ONE per unit: the exp stream gets PE priority
                # and the projection matmuls spread into the slack (the
                # 34-unit stream drains all 14 thunks well before their
                # consumer chunks)
                if thunkq:
                    thunkq.pop(0)()
            emit_av(pending)
            while epiq:
                emit_epilogue()

    if legalize:
        _legalize_multiwaits(nc)
    nc.finalize()
    return nc


def _legalize_multiwaits(nc):
    """Hardware instruction structs in this walrus build accept at most ONE
    sync wait. Move all but the last wait onto single-wait same-engine NoOps
    inserted right before the instruction (engines execute in order)."""
    for fn in nc.m.functions:
        for blk in fn.blocks:
            insts = blk.instructions
            out = []
            for inst in insts:
                si = inst.sync_info
                if si is not None and si.on_wait and len(si.on_wait) >= 2:
                    waits = list(si.on_wait)
                    for w in waits[:-1]:
                        out.append(mybir.InstNoOp(
                            name=nc.get_next_instruction_name(),
                            engine=inst.engine,
                            bass_nofuse=True,
                            sync_info=mybir.SyncInfo(
                                on_wait=[w], on_update=[]),
                        ))
                    inst.sync_info = mybir.SyncInfo(
                        on_wait=[waits[-1]],
                        on_update=list(si.on_update or []))
                out.append(inst)
            insts[:] = out


_NC_CACHE = {}


def get_nc(legalize=True):
    key = ("nc", legalize)
    if key not in _NC_CACHE:
        _NC_CACHE[key] = build_nc(legalize)
    return _NC_CACHE[key]


def make_core_inputs(x, Wq, bq, Wk, bk, Wv, bv):
    """Per-core input maps (host-side sharding). bk dropped (softmax
    invariance); bv applied on the host."""
    import ml_dtypes

    s = 1.0 / math.sqrt(D)
    wq16 = (np.asarray(Wq, np.float32) * s).astype(ml_dtypes.bfloat16)
    wk16 = np.asarray(Wk, np.float32).astype(ml_dtypes.bfloat16)
    wv16 = np.asarray(Wv, np.float32).astype(ml_dtypes.bfloat16)

    # msk: cols [0:128) all NEG; [128:256) staircase 0 if j >= k else NEG
    kk = np.arange(128)[:, None]
    jj = np.arange(128)[None, :]
    stair = np.where(jj >= kk, 0.0, NEG).astype(np.float32)
    mskf = np.concatenate([np.full((D, 128), NEG, np.float32), stair], axis=1)
    identf = np.eye(D, dtype=np.float32)
    cb16 = np.concatenate(
        [wq16, wk16, wv16,
         mskf.astype(ml_dtypes.bfloat16), identf.astype(ml_dtypes.bfloat16)],
        axis=1)  # [D, 768] bf16

    cf32 = np.zeros((D, 132), np.float32)
    cf32[:, 0] = np.asarray(bq, np.float32) * s
    cf32[:, 1] = -SHIFT
    cf32[:, 4:132] = 1.0     # f32r all-ones stationary for diag l-matmuls
    ones8 = np.ones((D, 2 * D), ml_dtypes.float8_e4m3)

    x = np.asarray(x, dtype=np.float32)
    in_maps = []
    for core in range(8):
        b, h = core // 2, core % 2
        xb = x[b]
        tri = xb[h * HALF:(h + 1) * HALF]          # [2048, 128]
        rect_q = xb[HALF:]                         # [2048, 128]
        rect_kv = xb[h * 1024:(h + 1) * 1024]      # [1024, 128]
        xin = np.ascontiguousarray(
            np.concatenate([tri, rect_q, rect_kv], axis=0).T
        ).astype(ml_dtypes.bfloat16)               # [128, 5120]
        in_maps.append({"xin": xin, "cb16": cb16, "cf32": cf32,
                        "ones8": ones8})
    return in_maps


def merge_outputs(results, bv):
    """Gather per-core (oT, lv) into the full [B, T, D] output. The -4
    score shift scales o and l identically, so it cancels in o/l."""
    bv = np.asarray(bv, dtype=np.float32)
    out = np.empty((B, T, D), np.float32)
    for b in range(B):
        lo, hi = results[2 * b], results[2 * b + 1]
        lo_lv = np.asarray(lo["lv"]).reshape(NCHUNK, CHUNK)
        hi_lv = np.asarray(hi["lv"]).reshape(NCHUNK, CHUNK)
        O = np.zeros((T, D), np.float64)
        L = np.zeros(T, np.float64)
        O[:HALF] += lo["oT"][:, :HALF].T
        L[:HALF] += lo_lv[0:4].ravel()
        O[HALF:] += hi["oT"][:, :HALF].T
        L[HALF:] += hi_lv[0:4].ravel()
        O[HALF:] += lo["oT"][:, HALF:].T
        L[HALF:] += lo_lv[4:8].ravel()
        O[HALF:] += hi["oT"][:, HALF:].T
        L[HALF:] += hi_lv[4:8].ravel()
        out[b] = (O / L[:, None]).astype(np.float32) + bv
    return out


def run_per_core(nc, in_maps, threads=True):
    """Run the same single-core program on each NeuronCore with its own
    inputs (per-core dispatch; the cores share no collectives)."""
    import jax
    from concourse import bass2jax

    devices = jax.devices()[:len(in_maps)]

    def one(i):
        with jax.default_device(devices[i]):
            return bass2jax.run_bass_via_pjrt(nc, [in_maps[i]], n_cores=1)[0]

    if threads:
        from concurrent.futures import ThreadPoolExecutor
        first = one(0)
        with ThreadPoolExecutor(max_workers=7) as ex:
            rest = list(ex.map(one, range(1, len(in_maps))))
        return [first] + rest
    return [one(i) for i in range(len(in_maps))]


def kernel(x, Wq, bq, Wk, bk, Wv, bv, _trace=False):
    from concourse.bass_utils import axon_active, run_bass_kernel_spmd

    nc = get_nc()
    in_maps = make_core_inputs(x, Wq, bq, Wk, bk, Wv, bv)
    if axon_active():
        results = run_per_core(nc, in_maps)
    else:
        res = run_bass_kernel_spmd(nc, in_maps, list(range(8)), trace=_trace)
        kernel.last_result = res
        results = res.results
    out = merge_outputs(results, bv)
    return out
